# revision 27
# baseline (speedup 1.0000x reference)
"""Trainium2 Bass kernel for a 12-head self-attention block.

Reference computation (per batch b of 8):
    qkv = x @ w_qkv                      # (1024, 2304)
    q, k, v per head (12 heads, d=64)
    attn = softmax(q k^T / sqrt(64))
    ctx  = attn @ v                      # (1024, 768)
    y    = ctx @ w_proj + b_proj

Sharding: data parallel over the batch dim — batch b runs on core b.
Each core gets the full weights and its own x slice; no collectives.

Performance structure (v3): the PE clock is HAM-gated — any idle gap
drops it from 2.4 GHz to 1.2 GHz for ~30 us, so the PE instruction
stream is kept gapless:
  - S matmuls have K=64; a head pair's k^T/q^T live on disjoint
    partition halves, so the pair's two S matmuls issue back-to-back
    with row tile_positions (0,*)/(64,*) and run CONCURRENTLY on
    disjoint PE row-groups (~2x S throughput).  Pairs 0-4 run paired;
    pair 5 runs its heads serially so the final drain is single-head.
  - PV matmuls lag their S by three iterations; qk^T / V chunk-groups
    interleave as filler so the PE never waits on ScalarE's exp (the
    only exp engine, ~2.2us per paired iteration).
  - The projection's per-tile accumulation groups open early (bias +
    kc0..4) to bridge the last head's normalize-chain drain; kc=5
    closes once the last ctx tile lands.  Proj tiles alternate between
    the two PSUM pools so four groups are always in flight.
  - Softmax denominators ride as a ones-column in V (free: PV streams
    cost N columns regardless of M=65).
  - y is written to DRAM as bf16 (halves writeback; host casts back).
"""

import numpy as np

N = 1024          # tokens per batch (32*32)
C = 768           # model dim
NH = 12           # heads
D = 64            # head dim
NT = N // 128     # 8 token tiles
KC = C // 128     # 6 contraction tiles
NP = NH // 2      # 6 head pairs
SCALE = D ** -0.5
NCORES = 8
PAIR_S = False    # paired issue thrashes the PE weight buffer (+100ns/MM)

_CACHE = {}


def _build_nc():
    import concourse.bass as bass
    import concourse.tile as tile
    from concourse import bacc, mybir
    from concourse.masks import make_identity
    from collections import deque

    F32 = mybir.dt.float32
    BF16 = mybir.dt.bfloat16
    Exp = mybir.ActivationFunctionType.Exp

    nc = bacc.Bacc(None, target_bir_lowering=False)
    x = nc.declare_dram_parameter("x", [N, C], F32, isOutput=False)
    wqkv = nc.declare_dram_parameter("w_qkv", [C, 3 * C], F32, isOutput=False)
    wproj = nc.declare_dram_parameter("w_proj", [C, C], F32, isOutput=False)
    bproj = nc.declare_dram_parameter("b_proj", [1, C], F32, isOutput=False)
    y = nc.declare_dram_parameter("y", [N, C], BF16, isOutput=True)

    with tile.TileContext(nc) as tc:
        from contextlib import ExitStack

        with ExitStack() as ctx:
            persist = ctx.enter_context(tc.tile_pool(name="persist", bufs=1))
            xT = persist.tile([128, KC, N], BF16)           # X^T (c, n)
            wqk = persist.tile([128, KC, NP, 2, 128], BF16)  # W_q|W_k per pair
            wv = persist.tile([128, KC, C], BF16)
            V = persist.tile([128, NT, NH, D + 2], BF16)    # v + ones col
            wp = persist.tile([128, KC, C], BF16)
            ctxT = persist.tile([128, KC, N], BF16)         # normalized ctx^T
            qkT = persist.tile([128, NP, 2, N], BF16)       # all pairs q^T/k^T
            ident = persist.tile([128, 128], BF16)
            ones_f32 = persist.tile([128, 128], F32)
            bias_sb = persist.tile([1, C], F32)
            bias_bc = persist.tile([128, C], F32)   # bias broadcast to 128 rows

            make_identity(nc, ident)
            nc.vector.memset(ones_f32[:], 1.0)
            for _t in range(NT):
                # ones written in pairs (4-byte chunks): lone 2-byte strided
                # writes are not safe on the compute engines
                nc.any.tensor_copy(
                    out=V[:, _t, :, D:D + 2],
                    in_=ones_f32[:, 0:2 * NH].rearrange(
                        "p (h two) -> p h two", two=2
                    ),
                )

            # ---- input DMAs ------------------------------------------------
            # x rows on the two HWDGE queues; weights on SWDGE (casts f32 ->
            # bf16 in flight).  SWDGE order = need order: pair-0 qk weights,
            # then wv (first PV needs V(t0)), remaining pairs, wproj.
            # wqkv viewed as [p, kc, {q,k,v}, pair, 128]
            wqkv_v = wqkv.rearrange(
                "(kc p) (three pair c) -> p kc three pair c",
                p=128, three=3, c=128,
            )
            for kc in range(0, KC, 3):
                nc.gpsimd.dma_start(
                    out=wv[:, kc:kc + 3, :],
                    in_=wqkv.rearrange("(kc p) n -> p kc n", p=128)[
                        :, kc:kc + 3, 2 * C:3 * C
                    ],
                )
            for qk_i in range(2):
                nc.gpsimd.dma_start(
                    out=wqk[:, :, 0, qk_i], in_=wqkv_v[:, :, qk_i, 0]
                )

            # x rows on the two HWDGE queues (SWDGE is busy with weights and
            # would delay the first tiles)
            xpool = ctx.enter_context(tc.tile_pool(name="xload", bufs=8))
            xin = []
            for nt in range(NT):
                xt_in = xpool.tile([128, C], F32, tag="x")
                r = slice(nt * 128, (nt + 1) * 128)
                nc.sync.dma_start(out=xt_in[:, 0:384], in_=x[r, 0:384])
                nc.scalar.dma_start(out=xt_in[:, 384:C], in_=x[r, 384:C])
                xin.append(xt_in)
            for j in range(1, NP):
                for qk_i in range(2):
                    nc.gpsimd.dma_start(
                        out=wqk[:, :, j, qk_i], in_=wqkv_v[:, :, qk_i, j]
                    )
            for kc in range(KC):
                nc.gpsimd.dma_start(
                    out=wp[:, kc, :],
                    in_=wproj[kc * 128:(kc + 1) * 128, :],
                )
            nc.gpsimd.dma_start(out=bias_sb[:], in_=bproj[:])
            nc.gpsimd.partition_broadcast(bias_bc[:], bias_sb[:], channels=128)

            # ---- PSUM pools (8 banks total, both pools 2 x 4KB slots) ------
            psA = ctx.enter_context(
                tc.tile_pool(name="psA", bufs=2, space="PSUM")
            )
            psO = ctx.enter_context(
                tc.tile_pool(name="psO", bufs=2, space="PSUM")
            )
            ptpool = ctx.enter_context(tc.tile_pool(name="pt", bufs=8))
            oupool = ctx.enter_context(tc.tile_pool(name="ou", bufs=2))
            bcpool = ctx.enter_context(tc.tile_pool(name="bc", bufs=2))
            outpool = ctx.enter_context(tc.tile_pool(name="out", bufs=6))

            # ---- Phase A: X^T transposes + pair-0 qk^T + V(t0..t3) ---------
            # x is cast f32->bf16 on DVE first so the PE transposes run at
            # 1 cycle/row instead of f32's 2
            xbpool = ctx.enter_context(tc.tile_pool(name="xb", bufs=3))

            def transpose_tile(nt):
                # cast per x-half so the first transposes start as soon as
                # the sync-queue half lands
                xb = xbpool.tile([128, C], BF16, tag="xb", name=f"xb{nt}")
                nc.vector.tensor_copy(out=xb[:, 0:384], in_=xin[nt][:, 0:384])
                nc.vector.tensor_copy(out=xb[:, 384:C], in_=xin[nt][:, 384:C])
                ps = psA.tile([128, KC, 128], BF16, tag="s", name=f"tp{nt}")
                for kc in range(KC):
                    nc.tensor.transpose(
                        ps[:, kc, :],
                        xb[:, kc * 128:(kc + 1) * 128],
                        ident[:],
                    )
                nc.vector.tensor_copy(
                    out=xT[:, :, nt * 128:(nt + 1) * 128], in_=ps[:]
                )

            def qk_unit(j, qk_i, cch):
                # one chunk-group: 128 channels x 512 tokens of q^T or k^T
                sl = slice(cch * 512, (cch + 1) * 512)
                ps = psA.tile(
                    [128, 512], F32, tag="s", name=f"qk{j}_{qk_i}_{cch}"
                )
                for kc in range(KC):
                    nc.tensor.matmul(
                        ps[:],
                        lhsT=wqk[:, kc, j, qk_i, :],
                        rhs=xT[:, kc, sl],
                        start=(kc == 0),
                        stop=(kc == KC - 1),
                    )
                nc.vector.tensor_copy(out=qkT[:, j, qk_i, sl], in_=ps[:])

            def v_unit(t, cch):
                # one chunk-group of V = X @ W_v (natural layout);
                # cch 0 covers heads 0..7, cch 1 heads 8..11
                sl = (slice(0, 512), slice(512, C))[cch]
                hs = (slice(0, 8), slice(8, NH))[cch]
                w = 512 if cch == 0 else C - 512
                ps = psA.tile([128, w], F32, tag="s", name=f"v{t}_{cch}")
                for kc in range(KC):
                    nc.tensor.matmul(
                        ps[:],
                        lhsT=xT[:, kc, t * 128:(t + 1) * 128],
                        rhs=wv[:, kc, sl],
                        start=(kc == 0),
                        stop=(kc == KC - 1),
                    )
                nc.vector.tensor_copy(
                    out=V[:, t, hs, 0:D],
                    in_=ps[:].rearrange("p (h d) -> p h d", d=D),
                )

            def qk_unit0(qk_i, q0):
                # pair-0 sub-chunk (256 cols): interleaves between transposes
                # so qk^T starts as soon as the first x tiles land
                ps = psA.tile(
                    [128, 256], F32, tag="s", name=f"qk0_{qk_i}_{q0}"
                )
                for kc in range(KC):
                    nc.tensor.matmul(
                        ps[:],
                        lhsT=wqk[:, kc, 0, qk_i, :],
                        rhs=xT[:, kc, q0:q0 + 256],
                        start=(kc == 0),
                        stop=(kc == KC - 1),
                    )
                nc.vector.tensor_copy(
                    out=qkT[:, 0, qk_i, q0:q0 + 256], in_=ps[:]
                )

            for nt in range(NT):
                transpose_tile(nt)
                if nt % 2 == 1:
                    q0 = (nt // 2) * 256
                    qk_unit0(0, q0)
                    qk_unit0(1, q0)
            for t in range(4):
                v_unit(t, 0)
                v_unit(t, 1)

            # Filler units, keyed by (pair, t) iteration of phase B; emitted
            # after that iteration's PVs.  Pair 0 hosts V(t4..7); pair j
            # hosts pair j+1's qk units.
            fill = {}

            def add_fill(j, t, fn):
                fill.setdefault((j, t), []).append(fn)

            for t in range(4, NT):
                it = 2 * (t - 4)
                add_fill(0, it, (lambda tt: lambda: v_unit(tt, 0))(t))
                add_fill(0, it + 1, (lambda tt: lambda: v_unit(tt, 1))(t))
            for j in range(1, NP):
                slots = ((0, 4), (0, 5), (0, 6), (0, 7)) if j == 1 else \
                    ((j - 1, 1), (j - 1, 3), (j - 1, 5), (j - 1, 7))
                for u, (jj, tt) in enumerate(slots):
                    qk_i, cch = divmod(u, 2)
                    add_fill(
                        jj, tt,
                        (lambda a, b, c: lambda: qk_unit(a, b, c))(j, qk_i, cch),
                    )

            # ---- Phase B: attention --------------------------------------
            OTs = {}

            def s_pair(j, t):
                # both heads' S tiles; with PAIR_S the two matmuls per chunk
                # sit on disjoint PE row-groups (K=64 at partitions 0/64) and
                # run concurrently
                S0 = psA.tile([128, N], F32, tag="s", name=f"s{2 * j}_{t}")
                S1 = psA.tile([128, N], F32, tag="s", name=f"s{2 * j + 1}_{t}")
                if PAIR_S:
                    order = [(cch, pb, S) for cch in range(2)
                             for pb, S in ((0, S0), (64, S1))]
                else:
                    order = [(cch, pb, S) for pb, S in ((0, S0), (64, S1))
                             for cch in range(2)]
                for cch, pb, S in order:
                    sl = slice(cch * 512, (cch + 1) * 512)
                    nc.tensor.matmul(
                        S[:, sl],
                        lhsT=qkT[pb:pb + 64, j, 1, t * 128:(t + 1) * 128],
                        rhs=qkT[pb:pb + 64, j, 0, sl],
                        start=True,
                        stop=True,
                    )
                pTs = []
                for hh, S in ((0, S0), (1, S1)):
                    pT = ptpool.tile(
                        [128, N], BF16, tag="pt", name=f"p{2 * j + hh}_{t}"
                    )
                    nc.scalar.activation(
                        out=pT[:], in_=S[:], func=Exp, scale=SCALE
                    )
                    pTs.append(pT)
                return pTs

            def s_single(h, t):
                j, hh = divmod(h, 2)
                pb = hh * 64
                S = psA.tile([128, N], F32, tag="s", name=f"s{h}_{t}")
                for cch in range(2):
                    sl = slice(cch * 512, (cch + 1) * 512)
                    nc.tensor.matmul(
                        S[:, sl],
                        lhsT=qkT[pb:pb + 64, j, 1, t * 128:(t + 1) * 128],
                        rhs=qkT[pb:pb + 64, j, 0, sl],
                        start=True,
                        stop=True,
                    )
                pT = ptpool.tile([128, N], BF16, tag="pt", name=f"p{h}_{t}")
                nc.scalar.activation(out=pT[:], in_=S[:], func=Exp, scale=SCALE)
                return pT

            def pv_matmul(h, t, pT):
                if t == 0:
                    OTs[h] = psO.tile([D + 1, N], F32, tag="ot", name=f"ot{h}")
                OT = OTs[h]
                for cch in range(2):
                    sl = slice(cch * 512, (cch + 1) * 512)
                    nc.tensor.matmul(
                        OT[:, sl],
                        lhsT=V[:, t, h, 0:D + 1],
                        rhs=pT[:, sl],
                        start=(t == 0),
                        stop=(t == NT - 1),
                    )

            def normalize(h, last=False):
                # copy O^T out fast (frees the PSUM slot), then recip the
                # denominator row (from a partition-0 tile — the custom-DVE
                # recip mis-executes on HW with a partition-base-64 input),
                # broadcast, multiply.
                j, hh = divmod(h, 2)
                pb = hh * 64
                OT = OTs.pop(h)
                if last:
                    # drain path: skip the ou stage, work straight from PSUM
                    # in column halves so proj closes unlock ASAP.  Den
                    # copies ride on ScalarE (idle here) so DVE's recip/mul
                    # chain isn't self-delayed.
                    den = bcpool.tile([1, N], F32, tag="den", name=f"d{h}")
                    bc = bcpool.tile([64, N], F32, tag="bc", name=f"b{h}")
                    for cch in range(2):
                        sl = slice(cch * 512, (cch + 1) * 512)
                        nc.scalar.copy(den[:, sl], OT[D:D + 1, sl])
                    for cch in range(2):
                        sl = slice(cch * 512, (cch + 1) * 512)
                        nc.vector.reciprocal_approx_fast(
                            out=bc[0:1, sl], in_=den[:, sl]
                        )
                        nc.gpsimd.partition_broadcast(
                            bc[:, sl], bc[0:1, sl], channels=64
                        )
                        nc.vector.tensor_mul(
                            out=ctxT[pb:pb + 64, j, sl],
                            in0=OT[0:D, sl],
                            in1=bc[:, sl],
                        )
                    return
                ou = oupool.tile([D + 1, N], F32, tag="ou", name=f"ou{h}")
                nc.vector.tensor_copy(out=ou[:], in_=OT[:])
                den = bcpool.tile([1, N], F32, tag="den", name=f"d{h}")
                nc.vector.tensor_copy(out=den[:], in_=ou[D:D + 1, :])
                bc = bcpool.tile([64, N], F32, tag="bc", name=f"b{h}")
                nc.vector.reciprocal_approx_fast(out=bc[0:1, :], in_=den[:])
                nc.gpsimd.partition_broadcast(bc[:], bc[0:1, :], channels=64)
                nc.vector.tensor_mul(
                    out=ctxT[pb:pb + 64, j, :], in0=ou[0:D, :], in1=bc[:]
                )

            # ---- Phase C: projection; first groups bridge the drain --------
            # group g = output tile nt=g//2, columns cch=g%2 (384 wide); one
            # [128, 2, 512] PSUM tile hosts two groups in bank-aligned halves.
            proj_ps = {}
            proj_ob = {}

            def proj_open(g, pool):
                nt, cch = divmod(g, 2)
                sl = slice(cch * 384, (cch + 1) * 384)
                if g % 2 == 0:
                    tag = "s" if pool is psA else "ot"
                    proj_ps[g // 2] = pool.tile(
                        [128, 2, 512], F32, tag=tag, name=f"pj{g // 2}"
                    )
                ps = proj_ps[g // 2][:, g % 2, 0:384]
                for kc in range(KC - 1):
                    nc.tensor.matmul(
                        ps,
                        lhsT=ctxT[:, kc, nt * 128:(nt + 1) * 128],
                        rhs=wp[:, kc, sl],
                        start=(kc == 0),
                        stop=False,
                    )

            def proj_close(g):
                nt, cch = divmod(g, 2)
                sl = slice(cch * 384, (cch + 1) * 384)
                ps = proj_ps[g // 2][:, g % 2, 0:384]
                nc.tensor.matmul(
                    ps,
                    lhsT=ctxT[:, KC - 1, nt * 128:(nt + 1) * 128],
                    rhs=wp[:, KC - 1, sl],
                    start=False,
                    stop=True,
                )
                # bias-add fused into the output cast on DVE (no bias matmul).
                # Even groups go via an ScalarE PSUM->SBUF copy first so the
                # PSUM slot recycles without waiting on the DVE queue.  Both
                # column halves land in one full-width ob tile so the y DMA
                # writes whole contiguous DRAM rows (1536B bursts, not 768B).
                if cch == 0:
                    proj_ob[nt] = outpool.tile(
                        [128, C], BF16, tag="ob", name=f"ob{nt}"
                    )
                ob = proj_ob[nt]
                if g % 2 == 0:
                    tmp = outpool.tile(
                        [128, 384], F32, tag="tmp", bufs=3, name=f"tm{g}"
                    )
                    nc.scalar.copy(tmp[:], ps)
                    src = tmp[:]
                else:
                    src = ps
                nc.vector.scalar_tensor_tensor(
                    out=ob[:, sl], in0=src, scalar=1.0, in1=bias_bc[:, sl],
                    op0=mybir.AluOpType.mult, op1=mybir.AluOpType.add,
                )
                if cch == 1:
                    r = slice(nt * 128, (nt + 1) * 128)
                    e0, e1 = ((nc.sync, nc.scalar), (nc.scalar, nc.gpsimd),
                              (nc.gpsimd, nc.sync))[nt % 3]
                    e0.dma_start(out=y[r, 0:384], in_=ob[:, 0:384])
                    e1.dma_start(out=y[r, 384:C], in_=ob[:, 384:C])

            # bridge: h11's exp + normalize chain runs under proj partials

            pending = deque()

            def pop_pv(last=False):
                h, t, pT = pending.popleft()
                pv_matmul(h, t, pT)
                if t == NT - 1:
                    normalize(h, last=last)

            for j in range(5):          # paired pairs 0..4, PV lag 3 iters
                for t in range(NT):
                    pT0, pT1 = s_pair(j, t)
                    pending.append((2 * j, t, pT0))
                    pending.append((2 * j + 1, t, pT1))
                    while len(pending) > 6:
                        pop_pv()
                    for fn in fill.get((j, t), ()):
                        fn()
            for h in (10, 11):          # pair 5 serial, PV lag 1
                for t in range(NT):
                    pT = s_single(h, t)
                    pending.append((h, t, pT))
                    while len(pending) > 1:
                        pop_pv()
                    if h == 11 and t in (2, 3):
                        # ScalarE paces this stretch; feed the idle PE the
                        # first proj partials (psO slot freed by h10's
                        # normalize)
                        proj_open(t - 2, psO)

            # psO slot 1 still holds OT(h11) until the normalize muls read
            # it, so the groups borrowing that slot (6,7) open only after
            # the first closes
            proj_open(2, psA)
            proj_open(3, psA)
            pop_pv(last=True)           # PV(h11, t7) + normalize(h11)
            proj_open(4, psA)
            proj_open(5, psA)
            for g in range(4):
                proj_close(g)
            proj_open(6, psO)
            proj_open(7, psO)
            for g in range(4, 8):
                proj_close(g)
            for g in range(8, 16):
                proj_open(g, psA if (g // 2) % 2 == 0 else psO)
                proj_close(g)

    nc.finalize()
    return nc


def _get_nc():
    if "nc" not in _CACHE:
        _CACHE["nc"] = _build_nc()
    return _CACHE["nc"]


def _make_in_maps(x, w_qkv, w_proj, b_proj):
    B = x.shape[0]
    xb = np.ascontiguousarray(x.reshape(B, N, C).astype(np.float32))
    w_qkv = np.ascontiguousarray(w_qkv.astype(np.float32))
    w_proj = np.ascontiguousarray(w_proj.astype(np.float32))
    bp = np.ascontiguousarray(b_proj.reshape(1, C).astype(np.float32))
    return [
        {"x": xb[b], "w_qkv": w_qkv, "w_proj": w_proj, "b_proj": bp}
        for b in range(B)
    ]


def _run(in_maps, **kwargs):
    from concourse.bass_utils import run_bass_kernel_spmd

    nc = _get_nc()
    return run_bass_kernel_spmd(
        nc, in_maps, core_ids=list(range(NCORES)), **kwargs
    )


def kernel(x, w_qkv, w_proj, b_proj):
    B, H, W, _ = x.shape
    res = _run(_make_in_maps(x, w_qkv, w_proj, b_proj))
    out = np.stack(
        [np.asarray(res.results[b]["y"], dtype=np.float32) for b in range(B)]
    )
    return out.reshape(B, H, W, C)


# revision 28
# speedup vs baseline: 1.0271x; 1.0271x over previous
"""Trainium2 Bass kernel for a 12-head self-attention block.

Reference computation (per batch b of 8):
    qkv = x @ w_qkv                      # (1024, 2304)
    q, k, v per head (12 heads, d=64)
    attn = softmax(q k^T / sqrt(64))
    ctx  = attn @ v                      # (1024, 768)
    y    = ctx @ w_proj + b_proj

Sharding: data parallel over the batch dim — batch b runs on core b.
Each core gets the full weights and its own x slice; no collectives.

Performance structure (v3): the PE clock is HAM-gated — any idle gap
drops it from 2.4 GHz to 1.2 GHz for ~30 us, so the PE instruction
stream is kept gapless:
  - S matmuls have K=64; a head pair's k^T/q^T live on disjoint
    partition halves, so the pair's two S matmuls issue back-to-back
    with row tile_positions (0,*)/(64,*) and run CONCURRENTLY on
    disjoint PE row-groups (~2x S throughput).  Pairs 0-4 run paired;
    pair 5 runs its heads serially so the final drain is single-head.
  - PV matmuls lag their S by three iterations; qk^T / V chunk-groups
    interleave as filler so the PE never waits on ScalarE's exp (the
    only exp engine, ~2.2us per paired iteration).
  - The projection's per-tile accumulation groups open early (bias +
    kc0..4) to bridge the last head's normalize-chain drain; kc=5
    closes once the last ctx tile lands.  Proj tiles alternate between
    the two PSUM pools so four groups are always in flight.
  - Softmax denominators ride as a ones-column in V (free: PV streams
    cost N columns regardless of M=65).
  - y is written to DRAM as bf16 (halves writeback; host casts back).
"""

import numpy as np

N = 1024          # tokens per batch (32*32)
C = 768           # model dim
NH = 12           # heads
D = 64            # head dim
NT = N // 128     # 8 token tiles
KC = C // 128     # 6 contraction tiles
NP = NH // 2      # 6 head pairs
SCALE = D ** -0.5
NCORES = 8
PAIR_S = False    # paired issue thrashes the PE weight buffer (+100ns/MM)

_CACHE = {}


def _build_nc():
    import concourse.bass as bass
    import concourse.tile as tile
    from concourse import bacc, mybir
    from concourse.masks import make_identity
    from collections import deque

    F32 = mybir.dt.float32
    BF16 = mybir.dt.bfloat16
    Exp = mybir.ActivationFunctionType.Exp

    nc = bacc.Bacc(None, target_bir_lowering=False)
    x = nc.declare_dram_parameter("x", [N, C], F32, isOutput=False)
    wqkv = nc.declare_dram_parameter("w_qkv", [C, 3 * C], F32, isOutput=False)
    wproj = nc.declare_dram_parameter("w_proj", [C, C], F32, isOutput=False)
    bproj = nc.declare_dram_parameter("b_proj", [1, C], F32, isOutput=False)
    y = nc.declare_dram_parameter("y", [N, C], BF16, isOutput=True)

    with tile.TileContext(nc) as tc:
        from contextlib import ExitStack

        with ExitStack() as ctx:
            persist = ctx.enter_context(tc.tile_pool(name="persist", bufs=1))
            xT = persist.tile([128, KC, N], BF16)           # X^T (c, n)
            wqk = persist.tile([128, KC, NP, 2, 128], BF16)  # W_q|W_k per pair
            wv = persist.tile([128, KC, C], BF16)
            V = persist.tile([128, NT, NH, D + 2], BF16)    # v + ones col
            wp = persist.tile([128, KC, C], BF16)
            ctxT = persist.tile([128, KC, N], BF16)         # normalized ctx^T
            qkT = persist.tile([128, NP, 2, N], BF16)       # all pairs q^T/k^T
            ident = persist.tile([128, 128], BF16)
            ones_f32 = persist.tile([128, 128], F32)
            bias_sb = persist.tile([1, C], F32)
            bias_bc = persist.tile([128, C], F32)   # bias broadcast to 128 rows

            make_identity(nc, ident)
            nc.vector.memset(ones_f32[:], 1.0)
            for _t in range(NT):
                # ones written in pairs (4-byte chunks): lone 2-byte strided
                # writes are not safe on the compute engines
                nc.any.tensor_copy(
                    out=V[:, _t, :, D:D + 2],
                    in_=ones_f32[:, 0:2 * NH].rearrange(
                        "p (h two) -> p h two", two=2
                    ),
                )

            # ---- input DMAs ------------------------------------------------
            # x rows on the two HWDGE queues; weights on SWDGE (casts f32 ->
            # bf16 in flight).  SWDGE order = need order: pair-0 qk weights,
            # then wv (first PV needs V(t0)), remaining pairs, wproj.
            # wqkv viewed as [p, kc, {q,k,v}, pair, 128]
            wqkv_v = wqkv.rearrange(
                "(kc p) (three pair c) -> p kc three pair c",
                p=128, three=3, c=128,
            )
            for qk_i in range(2):
                nc.gpsimd.dma_start(
                    out=wqk[:, :, 0, qk_i], in_=wqkv_v[:, :, qk_i, 0]
                )

            # x rows on the two HWDGE queues (SWDGE is busy with weights and
            # would delay the first tiles)
            xpool = ctx.enter_context(tc.tile_pool(name="xload", bufs=8))
            xin = []
            for nt in range(NT):
                xt_in = xpool.tile([128, C], F32, tag="x")
                r = slice(nt * 128, (nt + 1) * 128)
                nc.sync.dma_start(out=xt_in[:, 0:384], in_=x[r, 0:384])
                nc.scalar.dma_start(out=xt_in[:, 384:C], in_=x[r, 384:C])
                xin.append(xt_in)
            for kc in range(KC):
                nc.gpsimd.dma_start(
                    out=wv[:, kc, :],
                    in_=wqkv[kc * 128:(kc + 1) * 128, 2 * C:3 * C],
                )
            for j in range(1, NP):
                for qk_i in range(2):
                    nc.gpsimd.dma_start(
                        out=wqk[:, :, j, qk_i], in_=wqkv_v[:, :, qk_i, j]
                    )
            for kc in range(KC):
                nc.gpsimd.dma_start(
                    out=wp[:, kc, :],
                    in_=wproj[kc * 128:(kc + 1) * 128, :],
                )
            nc.gpsimd.dma_start(out=bias_sb[:], in_=bproj[:])
            nc.gpsimd.partition_broadcast(bias_bc[:], bias_sb[:], channels=128)

            # ---- PSUM pools (8 banks total, both pools 2 x 4KB slots) ------
            psA = ctx.enter_context(
                tc.tile_pool(name="psA", bufs=2, space="PSUM")
            )
            psO = ctx.enter_context(
                tc.tile_pool(name="psO", bufs=2, space="PSUM")
            )
            ptpool = ctx.enter_context(tc.tile_pool(name="pt", bufs=8))
            oupool = ctx.enter_context(tc.tile_pool(name="ou", bufs=2))
            bcpool = ctx.enter_context(tc.tile_pool(name="bc", bufs=2))
            outpool = ctx.enter_context(tc.tile_pool(name="out", bufs=6))

            # ---- Phase A: X^T transposes + pair-0 qk^T + V(t0..t3) ---------
            # x is cast f32->bf16 on DVE first so the PE transposes run at
            # 1 cycle/row instead of f32's 2
            xbpool = ctx.enter_context(tc.tile_pool(name="xb", bufs=3))

            def transpose_tile(nt):
                # cast per x-half so the first transposes start as soon as
                # the sync-queue half lands
                xb = xbpool.tile([128, C], BF16, tag="xb", name=f"xb{nt}")
                nc.vector.tensor_copy(out=xb[:, 0:384], in_=xin[nt][:, 0:384])
                nc.vector.tensor_copy(out=xb[:, 384:C], in_=xin[nt][:, 384:C])
                ps = psA.tile([128, KC, 128], BF16, tag="s", name=f"tp{nt}")
                for kc in range(KC):
                    nc.tensor.transpose(
                        ps[:, kc, :],
                        xb[:, kc * 128:(kc + 1) * 128],
                        ident[:],
                    )
                nc.vector.tensor_copy(
                    out=xT[:, :, nt * 128:(nt + 1) * 128], in_=ps[:]
                )

            def qk_unit(j, qk_i, cch):
                # one chunk-group: 128 channels x 512 tokens of q^T or k^T
                sl = slice(cch * 512, (cch + 1) * 512)
                ps = psA.tile(
                    [128, 512], F32, tag="s", name=f"qk{j}_{qk_i}_{cch}"
                )
                for kc in range(KC):
                    nc.tensor.matmul(
                        ps[:],
                        lhsT=wqk[:, kc, j, qk_i, :],
                        rhs=xT[:, kc, sl],
                        start=(kc == 0),
                        stop=(kc == KC - 1),
                    )
                nc.vector.tensor_copy(out=qkT[:, j, qk_i, sl], in_=ps[:])

            def v_unit(t, cch):
                # one chunk-group of V = X @ W_v (natural layout);
                # cch 0 covers heads 0..7, cch 1 heads 8..11
                sl = (slice(0, 512), slice(512, C))[cch]
                hs = (slice(0, 8), slice(8, NH))[cch]
                w = 512 if cch == 0 else C - 512
                ps = psA.tile([128, w], F32, tag="s", name=f"v{t}_{cch}")
                for kc in range(KC):
                    nc.tensor.matmul(
                        ps[:],
                        lhsT=xT[:, kc, t * 128:(t + 1) * 128],
                        rhs=wv[:, kc, sl],
                        start=(kc == 0),
                        stop=(kc == KC - 1),
                    )
                nc.vector.tensor_copy(
                    out=V[:, t, hs, 0:D],
                    in_=ps[:].rearrange("p (h d) -> p h d", d=D),
                )

            def qk_unit0(qk_i, q0):
                # pair-0 sub-chunk (256 cols): interleaves between transposes
                # so qk^T starts as soon as the first x tiles land
                ps = psA.tile(
                    [128, 256], F32, tag="s", name=f"qk0_{qk_i}_{q0}"
                )
                for kc in range(KC):
                    nc.tensor.matmul(
                        ps[:],
                        lhsT=wqk[:, kc, 0, qk_i, :],
                        rhs=xT[:, kc, q0:q0 + 256],
                        start=(kc == 0),
                        stop=(kc == KC - 1),
                    )
                nc.vector.tensor_copy(
                    out=qkT[:, 0, qk_i, q0:q0 + 256], in_=ps[:]
                )

            for nt in range(NT):
                transpose_tile(nt)
                if nt % 2 == 1:
                    q0 = (nt // 2) * 256
                    qk_unit0(0, q0)
                    qk_unit0(1, q0)
            for t in range(4):
                v_unit(t, 0)
                v_unit(t, 1)

            # Filler units, keyed by (pair, t) iteration of phase B; emitted
            # after that iteration's PVs.  Pair 0 hosts V(t4..7); pair j
            # hosts pair j+1's qk units.
            fill = {}

            def add_fill(j, t, fn):
                fill.setdefault((j, t), []).append(fn)

            for t in range(4, NT):
                it = 2 * (t - 4)
                add_fill(0, it, (lambda tt: lambda: v_unit(tt, 0))(t))
                add_fill(0, it + 1, (lambda tt: lambda: v_unit(tt, 1))(t))
            for j in range(1, NP):
                slots = ((0, 4), (0, 5), (0, 6), (0, 7)) if j == 1 else \
                    ((j - 1, 1), (j - 1, 3), (j - 1, 5), (j - 1, 7))
                for u, (jj, tt) in enumerate(slots):
                    qk_i, cch = divmod(u, 2)
                    add_fill(
                        jj, tt,
                        (lambda a, b, c: lambda: qk_unit(a, b, c))(j, qk_i, cch),
                    )

            # ---- Phase B: attention --------------------------------------
            OTs = {}

            def s_pair(j, t):
                # both heads' S tiles; with PAIR_S the two matmuls per chunk
                # sit on disjoint PE row-groups (K=64 at partitions 0/64) and
                # run concurrently
                S0 = psA.tile([128, N], F32, tag="s", name=f"s{2 * j}_{t}")
                S1 = psA.tile([128, N], F32, tag="s", name=f"s{2 * j + 1}_{t}")
                if PAIR_S:
                    order = [(cch, pb, S) for cch in range(2)
                             for pb, S in ((0, S0), (64, S1))]
                else:
                    order = [(cch, pb, S) for pb, S in ((0, S0), (64, S1))
                             for cch in range(2)]
                for cch, pb, S in order:
                    sl = slice(cch * 512, (cch + 1) * 512)
                    nc.tensor.matmul(
                        S[:, sl],
                        lhsT=qkT[pb:pb + 64, j, 1, t * 128:(t + 1) * 128],
                        rhs=qkT[pb:pb + 64, j, 0, sl],
                        start=True,
                        stop=True,
                    )
                pTs = []
                for hh, S in ((0, S0), (1, S1)):
                    pT = ptpool.tile(
                        [128, N], BF16, tag="pt", name=f"p{2 * j + hh}_{t}"
                    )
                    nc.scalar.activation(
                        out=pT[:], in_=S[:], func=Exp, scale=SCALE
                    )
                    pTs.append(pT)
                return pTs

            def s_single(h, t):
                j, hh = divmod(h, 2)
                pb = hh * 64
                S = psA.tile([128, N], F32, tag="s", name=f"s{h}_{t}")
                for cch in range(2):
                    sl = slice(cch * 512, (cch + 1) * 512)
                    nc.tensor.matmul(
                        S[:, sl],
                        lhsT=qkT[pb:pb + 64, j, 1, t * 128:(t + 1) * 128],
                        rhs=qkT[pb:pb + 64, j, 0, sl],
                        start=True,
                        stop=True,
                    )
                pT = ptpool.tile([128, N], BF16, tag="pt", name=f"p{h}_{t}")
                nc.scalar.activation(out=pT[:], in_=S[:], func=Exp, scale=SCALE)
                return pT

            def pv_matmul(h, t, pT):
                if t == 0:
                    OTs[h] = psO.tile([D + 1, N], F32, tag="ot", name=f"ot{h}")
                OT = OTs[h]
                for cch in range(2):
                    sl = slice(cch * 512, (cch + 1) * 512)
                    nc.tensor.matmul(
                        OT[:, sl],
                        lhsT=V[:, t, h, 0:D + 1],
                        rhs=pT[:, sl],
                        start=(t == 0),
                        stop=(t == NT - 1),
                    )

            def normalize(h, last=False):
                # copy O^T out fast (frees the PSUM slot), then recip the
                # denominator row (from a partition-0 tile — the custom-DVE
                # recip mis-executes on HW with a partition-base-64 input),
                # broadcast, multiply.
                j, hh = divmod(h, 2)
                pb = hh * 64
                OT = OTs.pop(h)
                if last:
                    # drain path: skip the ou stage, work straight from PSUM
                    # in column halves so proj closes unlock ASAP.  Den
                    # copies ride on ScalarE (idle here) so DVE's recip/mul
                    # chain isn't self-delayed.
                    den = bcpool.tile([1, N], F32, tag="den", name=f"d{h}")
                    bc = bcpool.tile([64, N], F32, tag="bc", name=f"b{h}")
                    for cch in range(2):
                        sl = slice(cch * 512, (cch + 1) * 512)
                        nc.scalar.copy(den[:, sl], OT[D:D + 1, sl])
                    for cch in range(2):
                        sl = slice(cch * 512, (cch + 1) * 512)
                        nc.vector.reciprocal_approx_fast(
                            out=bc[0:1, sl], in_=den[:, sl]
                        )
                        nc.gpsimd.partition_broadcast(
                            bc[:, sl], bc[0:1, sl], channels=64
                        )
                        nc.vector.tensor_mul(
                            out=ctxT[pb:pb + 64, j, sl],
                            in0=OT[0:D, sl],
                            in1=bc[:, sl],
                        )
                    return
                ou = oupool.tile([D + 1, N], F32, tag="ou", name=f"ou{h}")
                nc.vector.tensor_copy(out=ou[:], in_=OT[:])
                den = bcpool.tile([1, N], F32, tag="den", name=f"d{h}")
                nc.vector.tensor_copy(out=den[:], in_=ou[D:D + 1, :])
                bc = bcpool.tile([64, N], F32, tag="bc", name=f"b{h}")
                nc.vector.reciprocal_approx_fast(out=bc[0:1, :], in_=den[:])
                nc.gpsimd.partition_broadcast(bc[:], bc[0:1, :], channels=64)
                nc.vector.tensor_mul(
                    out=ctxT[pb:pb + 64, j, :], in0=ou[0:D, :], in1=bc[:]
                )

            # ---- Phase C: projection; first groups bridge the drain --------
            # group g = output tile nt=g//2, columns cch=g%2 (384 wide); one
            # [128, 2, 512] PSUM tile hosts two groups in bank-aligned halves.
            proj_ps = {}
            proj_ob = {}

            def proj_open(g, pool):
                nt, cch = divmod(g, 2)
                sl = slice(cch * 384, (cch + 1) * 384)
                if g % 2 == 0:
                    tag = "s" if pool is psA else "ot"
                    proj_ps[g // 2] = pool.tile(
                        [128, 2, 512], F32, tag=tag, name=f"pj{g // 2}"
                    )
                ps = proj_ps[g // 2][:, g % 2, 0:384]
                for kc in range(KC - 1):
                    nc.tensor.matmul(
                        ps,
                        lhsT=ctxT[:, kc, nt * 128:(nt + 1) * 128],
                        rhs=wp[:, kc, sl],
                        start=(kc == 0),
                        stop=False,
                    )

            def proj_close(g):
                nt, cch = divmod(g, 2)
                sl = slice(cch * 384, (cch + 1) * 384)
                ps = proj_ps[g // 2][:, g % 2, 0:384]
                nc.tensor.matmul(
                    ps,
                    lhsT=ctxT[:, KC - 1, nt * 128:(nt + 1) * 128],
                    rhs=wp[:, KC - 1, sl],
                    start=False,
                    stop=True,
                )
                # bias-add fused into the output cast on DVE (no bias matmul).
                # Even groups go via an ScalarE PSUM->SBUF copy first so the
                # PSUM slot recycles without waiting on the DVE queue.  Both
                # column halves land in one full-width ob tile so the y DMA
                # writes whole contiguous DRAM rows (1536B bursts, not 768B).
                if cch == 0:
                    proj_ob[nt] = outpool.tile(
                        [128, C], BF16, tag="ob", name=f"ob{nt}"
                    )
                ob = proj_ob[nt]
                if g % 2 == 0:
                    tmp = outpool.tile(
                        [128, 384], F32, tag="tmp", bufs=3, name=f"tm{g}"
                    )
                    nc.scalar.copy(tmp[:], ps)
                    src = tmp[:]
                else:
                    src = ps
                nc.vector.scalar_tensor_tensor(
                    out=ob[:, sl], in0=src, scalar=1.0, in1=bias_bc[:, sl],
                    op0=mybir.AluOpType.mult, op1=mybir.AluOpType.add,
                )
                if cch == 1:
                    r = slice(nt * 128, (nt + 1) * 128)
                    e0, e1 = ((nc.sync, nc.scalar), (nc.scalar, nc.gpsimd),
                              (nc.gpsimd, nc.sync))[nt % 3]
                    e0.dma_start(out=y[r, 0:384], in_=ob[:, 0:384])
                    e1.dma_start(out=y[r, 384:C], in_=ob[:, 384:C])

            # bridge: h11's exp + normalize chain runs under proj partials

            pending = deque()

            def pop_pv(last=False):
                h, t, pT = pending.popleft()
                pv_matmul(h, t, pT)
                if t == NT - 1:
                    normalize(h, last=last)

            for j in range(5):          # paired pairs 0..4, PV lag 3 iters
                for t in range(NT):
                    pT0, pT1 = s_pair(j, t)
                    pending.append((2 * j, t, pT0))
                    pending.append((2 * j + 1, t, pT1))
                    while len(pending) > 6:
                        pop_pv()
                    for fn in fill.get((j, t), ()):
                        fn()
            for h in (10, 11):          # pair 5 serial, PV lag 1
                for t in range(NT):
                    pT = s_single(h, t)
                    pending.append((h, t, pT))
                    while len(pending) > 1:
                        pop_pv()
                    if h == 11 and t in (2, 3):
                        # ScalarE paces this stretch; feed the idle PE the
                        # first proj partials (psO slot freed by h10's
                        # normalize)
                        proj_open(t - 2, psO)

            # psO slot 1 still holds OT(h11) until the normalize muls read
            # it, so the groups borrowing that slot (6,7) open only after
            # the first closes
            proj_open(2, psA)
            proj_open(3, psA)
            pop_pv(last=True)           # PV(h11, t7) + normalize(h11)
            proj_open(4, psA)
            proj_open(5, psA)
            for g in range(4):
                proj_close(g)
            proj_open(6, psO)
            proj_open(7, psO)
            for g in range(4, 8):
                proj_close(g)
            for g in range(8, 16):
                proj_open(g, psA if (g // 2) % 2 == 0 else psO)
                proj_close(g)

    nc.finalize()
    return nc


def _get_nc():
    if "nc" not in _CACHE:
        _CACHE["nc"] = _build_nc()
    return _CACHE["nc"]


def _make_in_maps(x, w_qkv, w_proj, b_proj):
    B = x.shape[0]
    xb = np.ascontiguousarray(x.reshape(B, N, C).astype(np.float32))
    w_qkv = np.ascontiguousarray(w_qkv.astype(np.float32))
    w_proj = np.ascontiguousarray(w_proj.astype(np.float32))
    bp = np.ascontiguousarray(b_proj.reshape(1, C).astype(np.float32))
    return [
        {"x": xb[b], "w_qkv": w_qkv, "w_proj": w_proj, "b_proj": bp}
        for b in range(B)
    ]


def _run(in_maps, **kwargs):
    from concourse.bass_utils import run_bass_kernel_spmd

    nc = _get_nc()
    return run_bass_kernel_spmd(
        nc, in_maps, core_ids=list(range(NCORES)), **kwargs
    )


def kernel(x, w_qkv, w_proj, b_proj):
    B, H, W, _ = x.shape
    res = _run(_make_in_maps(x, w_qkv, w_proj, b_proj))
    out = np.stack(
        [np.asarray(res.results[b]["y"], dtype=np.float32) for b in range(B)]
    )
    return out.reshape(B, H, W, C)


# revision 29
# speedup vs baseline: 1.2382x; 1.2055x over previous
"""Trainium2 Bass kernel for a 12-head self-attention block.

Reference computation (per batch b of 8):
    qkv = x @ w_qkv                      # (1024, 2304)
    q, k, v per head (12 heads, d=64)
    attn = softmax(q k^T / sqrt(64))
    ctx  = attn @ v                      # (1024, 768)
    y    = ctx @ w_proj + b_proj

Sharding: data parallel over the batch dim — batch b runs on core b.
Each core gets the full weights and its own x slice; no collectives.

Performance structure (v3): the PE clock is HAM-gated — any idle gap
drops it from 2.4 GHz to 1.2 GHz for ~30 us, so the PE instruction
stream is kept gapless:
  - S matmuls have K=64; a head pair's k^T/q^T live on disjoint
    partition halves, so the pair's two S matmuls issue back-to-back
    with row tile_positions (0,*)/(64,*) and run CONCURRENTLY on
    disjoint PE row-groups (~2x S throughput).  Pairs 0-4 run paired;
    pair 5 runs its heads serially so the final drain is single-head.
  - PV matmuls lag their S by three iterations; qk^T / V chunk-groups
    interleave as filler so the PE never waits on ScalarE's exp (the
    only exp engine, ~2.2us per paired iteration).
  - The projection's per-tile accumulation groups open early (bias +
    kc0..4) to bridge the last head's normalize-chain drain; kc=5
    closes once the last ctx tile lands.  Proj tiles alternate between
    the two PSUM pools so four groups are always in flight.
  - Softmax denominators ride as a ones-column in V (free: PV streams
    cost N columns regardless of M=65).
  - y is written to DRAM as bf16 (halves writeback; host casts back).
"""

import numpy as np

N = 1024          # tokens per batch (32*32)
C = 768           # model dim
NH = 12           # heads
D = 64            # head dim
NT = N // 128     # 8 token tiles
KC = C // 128     # 6 contraction tiles
NP = NH // 2      # 6 head pairs
SCALE = D ** -0.5
NCORES = 8
PAIR_S = False    # paired issue thrashes the PE weight buffer (+100ns/MM)

_CACHE = {}


def _build_nc():
    import concourse.bass as bass
    import concourse.tile as tile
    from concourse import bacc, mybir
    from concourse.masks import make_identity
    from collections import deque

    F32 = mybir.dt.float32
    BF16 = mybir.dt.bfloat16
    Exp = mybir.ActivationFunctionType.Exp

    nc = bacc.Bacc(None, target_bir_lowering=False)
    x = nc.declare_dram_parameter("x", [N, C], F32, isOutput=False)
    wqkv = nc.declare_dram_parameter("w_qkv", [C, 3 * C], F32, isOutput=False)
    wproj = nc.declare_dram_parameter("w_proj", [C, C], F32, isOutput=False)
    bproj = nc.declare_dram_parameter("b_proj", [1, C], F32, isOutput=False)
    y = nc.declare_dram_parameter("y", [N, C], BF16, isOutput=True)

    with tile.TileContext(nc) as tc:
        from contextlib import ExitStack

        with ExitStack() as ctx:
            persist = ctx.enter_context(tc.tile_pool(name="persist", bufs=1))
            xT = persist.tile([128, KC, N], BF16)           # X^T (c, n)
            wqk = persist.tile([128, KC, NP, 2, 128], BF16)  # W_q|W_k per pair
            wv = persist.tile([128, KC, C], BF16)
            V = persist.tile([128, NT, NH, D + 2], BF16)    # v + ones col
            wp = persist.tile([128, KC, C], BF16)
            ctxT = persist.tile([128, KC, N], BF16)         # normalized ctx^T
            qkT = persist.tile([128, NP, 2, N], BF16)       # all pairs q^T/k^T
            ident = persist.tile([128, 128], BF16)
            ones_f32 = persist.tile([128, 128], F32)
            bias_sb = persist.tile([1, C], F32)
            bias_bc = persist.tile([128, C], F32)   # bias broadcast to 128 rows

            make_identity(nc, ident)
            nc.vector.memset(ones_f32[:], 1.0)
            for _t in range(NT):
                # ones written in pairs (4-byte chunks): lone 2-byte strided
                # writes are not safe on the compute engines
                nc.any.tensor_copy(
                    out=V[:, _t, :, D:D + 2],
                    in_=ones_f32[:, 0:2 * NH].rearrange(
                        "p (h two) -> p h two", two=2
                    ),
                )

            # ---- input DMAs ------------------------------------------------
            # x rows on the two HWDGE queues; weights on SWDGE (casts f32 ->
            # bf16 in flight).  SWDGE order = need order: pair-0 qk weights,
            # then wv (first PV needs V(t0)), remaining pairs, wproj.
            # wqkv viewed as [p, kc, {q,k,v}, pair, 128]
            wqkv_v = wqkv.rearrange(
                "(kc p) (three pair c) -> p kc three pair c",
                p=128, three=3, c=128,
            )
            for qk_i in range(2):
                nc.gpsimd.dma_start(
                    out=wqk[:, :, 0, qk_i], in_=wqkv_v[:, :, qk_i, 0]
                )

            # x rows on the two HWDGE queues (SWDGE is busy with weights and
            # would delay the first tiles)
            xpool = ctx.enter_context(tc.tile_pool(name="xload", bufs=8))
            xin = []
            for nt in range(NT):
                xt_in = xpool.tile([128, C], F32, tag="x")
                r = slice(nt * 128, (nt + 1) * 128)
                nc.sync.dma_start(out=xt_in[:, 0:384], in_=x[r, 0:384])
                nc.scalar.dma_start(out=xt_in[:, 384:C], in_=x[r, 384:C])
                xin.append(xt_in)
            for kc in range(KC):
                nc.gpsimd.dma_start(
                    out=wv[:, kc, :],
                    in_=wqkv[kc * 128:(kc + 1) * 128, 2 * C:3 * C],
                )
            for j in range(1, NP):
                for qk_i in range(2):
                    nc.gpsimd.dma_start(
                        out=wqk[:, :, j, qk_i], in_=wqkv_v[:, :, qk_i, j]
                    )
            for kc in range(KC):
                nc.gpsimd.dma_start(
                    out=wp[:, kc, :],
                    in_=wproj[kc * 128:(kc + 1) * 128, :],
                )
            nc.gpsimd.dma_start(out=bias_sb[:], in_=bproj[:])
            nc.gpsimd.partition_broadcast(bias_bc[:], bias_sb[:], channels=128)

            # ---- PSUM pools (8 banks total, both pools 2 x 4KB slots) ------
            psA = ctx.enter_context(
                tc.tile_pool(name="psA", bufs=2, space="PSUM")
            )
            psO = ctx.enter_context(
                tc.tile_pool(name="psO", bufs=2, space="PSUM")
            )
            ptpool = ctx.enter_context(tc.tile_pool(name="pt", bufs=8))
            oupool = ctx.enter_context(tc.tile_pool(name="ou", bufs=2))
            bcpool = ctx.enter_context(tc.tile_pool(name="bc", bufs=2))
            outpool = ctx.enter_context(tc.tile_pool(name="out", bufs=6))

            # ---- Phase A: X^T transposes + pair-0 qk^T + V(t0..t3) ---------
            # x is cast f32->bf16 on DVE first so the PE transposes run at
            # 1 cycle/row instead of f32's 2
            xbpool = ctx.enter_context(tc.tile_pool(name="xb", bufs=3))

            def transpose_tile(nt):
                # cast per x-half so the first transposes start as soon as
                # the sync-queue half lands
                xb = xbpool.tile([128, C], BF16, tag="xb", name=f"xb{nt}")
                nc.vector.tensor_copy(out=xb[:, 0:384], in_=xin[nt][:, 0:384])
                nc.vector.tensor_copy(out=xb[:, 384:C], in_=xin[nt][:, 384:C])
                ps = psA.tile([128, KC, 128], BF16, tag="s", name=f"tp{nt}")
                for kc in range(KC):
                    nc.tensor.transpose(
                        ps[:, kc, :],
                        xb[:, kc * 128:(kc + 1) * 128],
                        ident[:],
                    )
                nc.vector.tensor_copy(
                    out=xT[:, :, nt * 128:(nt + 1) * 128], in_=ps[:]
                )

            def qk_unit(j, qk_i, cch):
                # one chunk-group: 128 channels x 512 tokens of q^T or k^T
                sl = slice(cch * 512, (cch + 1) * 512)
                ps = psA.tile(
                    [128, 512], F32, tag="s", name=f"qk{j}_{qk_i}_{cch}"
                )
                for kc in range(KC):
                    nc.tensor.matmul(
                        ps[:],
                        lhsT=wqk[:, kc, j, qk_i, :],
                        rhs=xT[:, kc, sl],
                        start=(kc == 0),
                        stop=(kc == KC - 1),
                    )
                nc.vector.tensor_copy(out=qkT[:, j, qk_i, sl], in_=ps[:])

            def v_unit(t, cch):
                # one chunk-group of V = X @ W_v (natural layout);
                # cch 0 covers heads 0..7, cch 1 heads 8..11
                sl = (slice(0, 512), slice(512, C))[cch]
                hs = (slice(0, 8), slice(8, NH))[cch]
                w = 512 if cch == 0 else C - 512
                ps = psA.tile([128, w], F32, tag="s", name=f"v{t}_{cch}")
                for kc in range(KC):
                    nc.tensor.matmul(
                        ps[:],
                        lhsT=xT[:, kc, t * 128:(t + 1) * 128],
                        rhs=wv[:, kc, sl],
                        start=(kc == 0),
                        stop=(kc == KC - 1),
                    )
                nc.vector.tensor_copy(
                    out=V[:, t, hs, 0:D],
                    in_=ps[:].rearrange("p (h d) -> p h d", d=D),
                )

            def qk_unit0(qk_i, q0):
                # pair-0 sub-chunk (256 cols): interleaves between transposes
                # so qk^T starts as soon as the first x tiles land
                ps = psA.tile(
                    [128, 256], F32, tag="s", name=f"qk0_{qk_i}_{q0}"
                )
                for kc in range(KC):
                    nc.tensor.matmul(
                        ps[:],
                        lhsT=wqk[:, kc, 0, qk_i, :],
                        rhs=xT[:, kc, q0:q0 + 256],
                        start=(kc == 0),
                        stop=(kc == KC - 1),
                    )
                nc.vector.tensor_copy(
                    out=qkT[:, 0, qk_i, q0:q0 + 256], in_=ps[:]
                )

            for nt in range(NT):
                transpose_tile(nt)
                if nt % 2 == 1:
                    q0 = (nt // 2) * 256
                    qk_unit0(0, q0)
                    qk_unit0(1, q0)
            for t in range(4):
                v_unit(t, 0)
                v_unit(t, 1)

            # Filler units, keyed by (pair, t) iteration of phase B; emitted
            # after that iteration's PVs.  Pair 0 hosts V(t4..7); pair j
            # hosts pair j+1's qk units.
            fill = {}

            def add_fill(j, t, fn):
                fill.setdefault((j, t), []).append(fn)

            for t in range(4, NT):
                it = 2 * (t - 4)
                add_fill(0, it, (lambda tt: lambda: v_unit(tt, 0))(t))
                add_fill(0, it + 1, (lambda tt: lambda: v_unit(tt, 1))(t))
            for j in range(1, NP):
                slots = ((0, 4), (0, 5), (0, 6), (0, 7)) if j == 1 else \
                    ((j - 1, 1), (j - 1, 3), (j - 1, 5), (j - 1, 7))
                for u, (jj, tt) in enumerate(slots):
                    qk_i, cch = divmod(u, 2)
                    add_fill(
                        jj, tt,
                        (lambda a, b, c: lambda: qk_unit(a, b, c))(j, qk_i, cch),
                    )

            # ---- Phase B: attention --------------------------------------
            OTs = {}

            def s_pair(j, t):
                # both heads' S tiles; with PAIR_S the two matmuls per chunk
                # sit on disjoint PE row-groups (K=64 at partitions 0/64) and
                # run concurrently
                S0 = psA.tile([128, N], F32, tag="s", name=f"s{2 * j}_{t}")
                S1 = psA.tile([128, N], F32, tag="s", name=f"s{2 * j + 1}_{t}")
                if PAIR_S:
                    order = [(cch, pb, S) for cch in range(2)
                             for pb, S in ((0, S0), (64, S1))]
                else:
                    order = [(cch, pb, S) for pb, S in ((0, S0), (64, S1))
                             for cch in range(2)]
                for cch, pb, S in order:
                    sl = slice(cch * 512, (cch + 1) * 512)
                    nc.tensor.matmul(
                        S[:, sl],
                        lhsT=qkT[pb:pb + 64, j, 1, t * 128:(t + 1) * 128],
                        rhs=qkT[pb:pb + 64, j, 0, sl],
                        start=True,
                        stop=True,
                    )
                pTs = []
                for hh, S in ((0, S0), (1, S1)):
                    pT = ptpool.tile(
                        [128, N], BF16, tag="pt", name=f"p{2 * j + hh}_{t}"
                    )
                    nc.scalar.activation(
                        out=pT[:], in_=S[:], func=Exp, scale=SCALE
                    )
                    pTs.append(pT)
                return pTs

            def s_single(h, t):
                j, hh = divmod(h, 2)
                pb = hh * 64
                S = psA.tile([128, N], F32, tag="s", name=f"s{h}_{t}")
                for cch in range(2):
                    sl = slice(cch * 512, (cch + 1) * 512)
                    nc.tensor.matmul(
                        S[:, sl],
                        lhsT=qkT[pb:pb + 64, j, 1, t * 128:(t + 1) * 128],
                        rhs=qkT[pb:pb + 64, j, 0, sl],
                        start=True,
                        stop=True,
                    )
                pT = ptpool.tile([128, N], BF16, tag="pt", name=f"p{h}_{t}")
                nc.scalar.activation(out=pT[:], in_=S[:], func=Exp, scale=SCALE)
                return pT

            def pv_matmul(h, t, pT):
                if t == 0:
                    OTs[h] = psO.tile([D + 1, N], F32, tag="ot", name=f"ot{h}")
                OT = OTs[h]
                for cch in range(2):
                    sl = slice(cch * 512, (cch + 1) * 512)
                    nc.tensor.matmul(
                        OT[:, sl],
                        lhsT=V[:, t, h, 0:D + 1],
                        rhs=pT[:, sl],
                        start=(t == 0),
                        stop=(t == NT - 1),
                    )

            def normalize(h, last=False):
                # copy O^T out fast (frees the PSUM slot), then recip the
                # denominator row (from a partition-0 tile — the custom-DVE
                # recip mis-executes on HW with a partition-base-64 input),
                # broadcast, multiply.
                j, hh = divmod(h, 2)
                pb = hh * 64
                OT = OTs.pop(h)
                if last:
                    # drain path: skip the ou stage, work straight from PSUM
                    # in column halves so proj closes unlock ASAP.  Den
                    # copies ride on ScalarE (idle here) so DVE's recip/mul
                    # chain isn't self-delayed.
                    den = bcpool.tile([1, N], F32, tag="den", name=f"d{h}")
                    bc = bcpool.tile([64, N], F32, tag="bc", name=f"b{h}")
                    for cch in range(2):
                        sl = slice(cch * 512, (cch + 1) * 512)
                        nc.scalar.copy(den[:, sl], OT[D:D + 1, sl])
                    for cch in range(2):
                        sl = slice(cch * 512, (cch + 1) * 512)
                        nc.vector.reciprocal_approx_fast(
                            out=bc[0:1, sl], in_=den[:, sl]
                        )
                        nc.gpsimd.partition_broadcast(
                            bc[:, sl], bc[0:1, sl], channels=64
                        )
                        nc.vector.tensor_mul(
                            out=ctxT[pb:pb + 64, j, sl],
                            in0=OT[0:D, sl],
                            in1=bc[:, sl],
                        )
                    return
                ou = oupool.tile([D + 1, N], F32, tag="ou", name=f"ou{h}")
                nc.vector.tensor_copy(out=ou[:], in_=OT[:])
                den = bcpool.tile([1, N], F32, tag="den", name=f"d{h}")
                nc.vector.tensor_copy(out=den[:], in_=ou[D:D + 1, :])
                bc = bcpool.tile([64, N], F32, tag="bc", name=f"b{h}")
                nc.vector.reciprocal_approx_fast(out=bc[0:1, :], in_=den[:])
                nc.gpsimd.partition_broadcast(bc[:], bc[0:1, :], channels=64)
                nc.vector.tensor_mul(
                    out=ctxT[pb:pb + 64, j, :], in0=ou[0:D, :], in1=bc[:]
                )

            # ---- Phase C: projection; first groups bridge the drain --------
            # group g = output tile nt=g//2, columns cch=g%2 (384 wide); one
            # [128, 2, 512] PSUM tile hosts two groups in bank-aligned halves.
            proj_ps = {}
            proj_ob = {}

            def proj_open(g, pool):
                nt, cch = divmod(g, 2)
                sl = slice(cch * 384, (cch + 1) * 384)
                if g % 2 == 0:
                    tag = "s" if pool is psA else "ot"
                    proj_ps[g // 2] = pool.tile(
                        [128, 2, 512], F32, tag=tag, name=f"pj{g // 2}"
                    )
                ps = proj_ps[g // 2][:, g % 2, 0:384]
                for kc in range(KC - 1):
                    nc.tensor.matmul(
                        ps,
                        lhsT=ctxT[:, kc, nt * 128:(nt + 1) * 128],
                        rhs=wp[:, kc, sl],
                        start=(kc == 0),
                        stop=False,
                    )

            def proj_close(g):
                nt, cch = divmod(g, 2)
                sl = slice(cch * 384, (cch + 1) * 384)
                ps = proj_ps[g // 2][:, g % 2, 0:384]
                nc.tensor.matmul(
                    ps,
                    lhsT=ctxT[:, KC - 1, nt * 128:(nt + 1) * 128],
                    rhs=wp[:, KC - 1, sl],
                    start=False,
                    stop=True,
                )
                # bias-add fused into the output cast on DVE (no bias matmul).
                # Even groups go via an ScalarE PSUM->SBUF copy first so the
                # PSUM slot recycles without waiting on the DVE queue.  Both
                # column halves land in one full-width ob tile so the y DMA
                # writes whole contiguous DRAM rows (1536B bursts, not 768B).
                if cch == 0:
                    proj_ob[nt] = outpool.tile(
                        [128, C], BF16, tag="ob", name=f"ob{nt}"
                    )
                ob = proj_ob[nt]
                if g % 2 == 0:
                    tmp = outpool.tile(
                        [128, 384], F32, tag="tmp", bufs=3, name=f"tm{g}"
                    )
                    nc.scalar.copy(tmp[:], ps)
                    src = tmp[:]
                else:
                    src = ps
                nc.vector.scalar_tensor_tensor(
                    out=ob[:, sl], in0=src, scalar=1.0, in1=bias_bc[:, sl],
                    op0=mybir.AluOpType.mult, op1=mybir.AluOpType.add,
                )
                if cch == 1:
                    eng = (nc.sync, nc.scalar, nc.gpsimd)[nt % 3]
                    eng.dma_start(
                        out=y[nt * 128:(nt + 1) * 128, :], in_=ob[:]
                    )

            # bridge: h11's exp + normalize chain runs under proj partials

            pending = deque()

            def pop_pv(last=False):
                h, t, pT = pending.popleft()
                pv_matmul(h, t, pT)
                if t == NT - 1:
                    normalize(h, last=last)

            for j in range(5):          # paired pairs 0..4, PV lag 3 iters
                for t in range(NT):
                    pT0, pT1 = s_pair(j, t)
                    pending.append((2 * j, t, pT0))
                    pending.append((2 * j + 1, t, pT1))
                    while len(pending) > 6:
                        pop_pv()
                    for fn in fill.get((j, t), ()):
                        fn()
            for h in (10, 11):          # pair 5 serial, PV lag 1
                for t in range(NT):
                    pT = s_single(h, t)
                    pending.append((h, t, pT))
                    while len(pending) > 1:
                        pop_pv()

            # psO slot 1 still holds OT(h11) until the normalize muls read
            # it, so the groups borrowing that slot (6,7) open only after
            # the first closes
            proj_open(0, psA)
            proj_open(1, psA)
            pop_pv(last=True)           # PV(h11, t7) + normalize(h11)
            proj_open(2, psO)
            proj_open(3, psO)
            proj_open(4, psA)
            proj_open(5, psA)
            for g in range(4):
                proj_close(g)
            proj_open(6, psO)
            proj_open(7, psO)
            for g in range(4, 8):
                proj_close(g)
            for g in range(8, 16):
                proj_open(g, psA if (g // 2) % 2 == 0 else psO)
                proj_close(g)

    nc.finalize()
    return nc


def _get_nc():
    if "nc" not in _CACHE:
        _CACHE["nc"] = _build_nc()
    return _CACHE["nc"]


def _make_in_maps(x, w_qkv, w_proj, b_proj):
    B = x.shape[0]
    xb = np.ascontiguousarray(x.reshape(B, N, C).astype(np.float32))
    w_qkv = np.ascontiguousarray(w_qkv.astype(np.float32))
    w_proj = np.ascontiguousarray(w_proj.astype(np.float32))
    bp = np.ascontiguousarray(b_proj.reshape(1, C).astype(np.float32))
    return [
        {"x": xb[b], "w_qkv": w_qkv, "w_proj": w_proj, "b_proj": bp}
        for b in range(B)
    ]


def _run(in_maps, **kwargs):
    from concourse.bass_utils import run_bass_kernel_spmd

    nc = _get_nc()
    return run_bass_kernel_spmd(
        nc, in_maps, core_ids=list(range(NCORES)), **kwargs
    )


def kernel(x, w_qkv, w_proj, b_proj):
    B, H, W, _ = x.shape
    res = _run(_make_in_maps(x, w_qkv, w_proj, b_proj))
    out = np.stack(
        [np.asarray(res.results[b]["y"], dtype=np.float32) for b in range(B)]
    )
    return out.reshape(B, H, W, C)


# revision 32
# speedup vs baseline: 1.2428x; 1.0038x over previous
"""Trainium2 Bass kernel for a 12-head self-attention block.

Reference computation (per batch b of 8):
    qkv = x @ w_qkv                      # (1024, 2304)
    q, k, v per head (12 heads, d=64)
    attn = softmax(q k^T / sqrt(64))
    ctx  = attn @ v                      # (1024, 768)
    y    = ctx @ w_proj + b_proj

Sharding: data parallel over the batch dim — batch b runs on core b.
Each core gets the full weights and its own x slice; no collectives.

Performance structure (v3): the PE clock is HAM-gated — any idle gap
drops it from 2.4 GHz to 1.2 GHz for ~30 us, so the PE instruction
stream is kept gapless:
  - S matmuls have K=64; a head pair's k^T/q^T live on disjoint
    partition halves, so the pair's two S matmuls issue back-to-back
    with row tile_positions (0,*)/(64,*) and run CONCURRENTLY on
    disjoint PE row-groups (~2x S throughput).  Pairs 0-4 run paired;
    pair 5 runs its heads serially so the final drain is single-head.
  - PV matmuls lag their S by three iterations; qk^T / V chunk-groups
    interleave as filler so the PE never waits on ScalarE's exp (the
    only exp engine, ~2.2us per paired iteration).
  - The projection's per-tile accumulation groups open early (bias +
    kc0..4) to bridge the last head's normalize-chain drain; kc=5
    closes once the last ctx tile lands.  Proj tiles alternate between
    the two PSUM pools so four groups are always in flight.
  - Softmax denominators ride as a ones-column in V (free: PV streams
    cost N columns regardless of M=65).
  - y is written to DRAM as bf16 (halves writeback; host casts back).
"""

import numpy as np

N = 1024          # tokens per batch (32*32)
C = 768           # model dim
NH = 12           # heads
D = 64            # head dim
NT = N // 128     # 8 token tiles
KC = C // 128     # 6 contraction tiles
NP = NH // 2      # 6 head pairs
SCALE = D ** -0.5
NCORES = 8
PAIR_S = False    # paired issue thrashes the PE weight buffer (+100ns/MM)

_CACHE = {}


def _build_nc():
    import concourse.bass as bass
    import concourse.tile as tile
    from concourse import bacc, mybir
    from concourse.masks import make_identity
    from collections import deque

    F32 = mybir.dt.float32
    BF16 = mybir.dt.bfloat16
    Exp = mybir.ActivationFunctionType.Exp

    nc = bacc.Bacc(None, target_bir_lowering=False)
    x = nc.declare_dram_parameter("x", [N, C], F32, isOutput=False)
    wqkv = nc.declare_dram_parameter("w_qkv", [C, 3 * C], F32, isOutput=False)
    wproj = nc.declare_dram_parameter("w_proj", [C, C], F32, isOutput=False)
    bproj = nc.declare_dram_parameter("b_proj", [1, C], F32, isOutput=False)
    y = nc.declare_dram_parameter("y", [N, C], BF16, isOutput=True)

    with tile.TileContext(nc) as tc:
        from contextlib import ExitStack

        with ExitStack() as ctx:
            persist = ctx.enter_context(tc.tile_pool(name="persist", bufs=1))
            xT = persist.tile([128, KC, N], BF16)           # X^T (c, n)
            wqk = persist.tile([128, KC, NP, 2, 128], BF16)  # W_q|W_k per pair
            wv = persist.tile([128, KC, C], BF16)
            V = persist.tile([128, NT, NH, D + 2], BF16)    # v + ones col
            wp = persist.tile([128, KC, C], BF16)
            ctxT = persist.tile([128, KC, N], BF16)         # normalized ctx^T
            qkT = persist.tile([128, NP, 2, N], BF16)       # all pairs q^T/k^T
            ident = persist.tile([128, 128], BF16)
            ones_f32 = persist.tile([128, 128], F32)
            bias_sb = persist.tile([1, C], F32)
            bias_bc = persist.tile([128, C], F32)   # bias broadcast to 128 rows

            make_identity(nc, ident)
            nc.vector.memset(ones_f32[:], 1.0)
            for _t in range(NT):
                # ones written in pairs (4-byte chunks): lone 2-byte strided
                # writes are not safe on the compute engines
                nc.any.tensor_copy(
                    out=V[:, _t, :, D:D + 2],
                    in_=ones_f32[:, 0:2 * NH].rearrange(
                        "p (h two) -> p h two", two=2
                    ),
                )

            # ---- input DMAs ------------------------------------------------
            # x rows on the two HWDGE queues; weights on SWDGE (casts f32 ->
            # bf16 in flight).  SWDGE order = need order: pair-0 qk weights,
            # then wv (first PV needs V(t0)), remaining pairs, wproj.
            # wqkv viewed as [p, kc, {q,k,v}, pair, 128]
            wqkv_v = wqkv.rearrange(
                "(kc p) (three pair c) -> p kc three pair c",
                p=128, three=3, c=128,
            )
            for qk_i in range(2):
                nc.gpsimd.dma_start(
                    out=wqk[:, :, 0, qk_i], in_=wqkv_v[:, :, qk_i, 0]
                )

            # x rows on the two HWDGE queues (SWDGE is busy with weights and
            # would delay the first tiles)
            xpool = ctx.enter_context(tc.tile_pool(name="xload", bufs=8))
            xin = []
            for nt in range(NT):
                xt_in = xpool.tile([128, C], F32, tag="x")
                r = slice(nt * 128, (nt + 1) * 128)
                nc.sync.dma_start(out=xt_in[:, 0:384], in_=x[r, 0:384])
                nc.scalar.dma_start(out=xt_in[:, 384:C], in_=x[r, 384:C])
                xin.append(xt_in)
            for kc in range(KC):
                nc.gpsimd.dma_start(
                    out=wv[:, kc, :],
                    in_=wqkv[kc * 128:(kc + 1) * 128, 2 * C:3 * C],
                )
            for j in range(1, NP):
                for qk_i in range(2):
                    nc.gpsimd.dma_start(
                        out=wqk[:, :, j, qk_i], in_=wqkv_v[:, :, qk_i, j]
                    )
            for kc in range(KC):
                nc.gpsimd.dma_start(
                    out=wp[:, kc, :],
                    in_=wproj[kc * 128:(kc + 1) * 128, :],
                )
            nc.gpsimd.dma_start(out=bias_sb[:], in_=bproj[:])
            nc.gpsimd.partition_broadcast(bias_bc[:], bias_sb[:], channels=128)

            # ---- PSUM pools (8 banks total, both pools 2 x 4KB slots) ------
            psA = ctx.enter_context(
                tc.tile_pool(name="psA", bufs=2, space="PSUM")
            )
            psO = ctx.enter_context(
                tc.tile_pool(name="psO", bufs=2, space="PSUM")
            )
            ptpool = ctx.enter_context(tc.tile_pool(name="pt", bufs=8))
            oupool = ctx.enter_context(tc.tile_pool(name="ou", bufs=2))
            bcpool = ctx.enter_context(tc.tile_pool(name="bc", bufs=2))
            outpool = ctx.enter_context(tc.tile_pool(name="out", bufs=6))

            # ---- Phase A: X^T transposes + pair-0 qk^T + V(t0..t3) ---------
            # x is cast f32->bf16 on DVE first so the PE transposes run at
            # 1 cycle/row instead of f32's 2
            xbpool = ctx.enter_context(tc.tile_pool(name="xb", bufs=3))

            def transpose_tile(nt):
                # cast per x-half so the first transposes start as soon as
                # the sync-queue half lands
                xb = xbpool.tile([128, C], BF16, tag="xb", name=f"xb{nt}")
                nc.vector.tensor_copy(out=xb[:, 0:384], in_=xin[nt][:, 0:384])
                nc.vector.tensor_copy(out=xb[:, 384:C], in_=xin[nt][:, 384:C])
                ps = psA.tile([128, KC, 128], BF16, tag="s", name=f"tp{nt}")
                for kc in range(KC):
                    nc.tensor.transpose(
                        ps[:, kc, :],
                        xb[:, kc * 128:(kc + 1) * 128],
                        ident[:],
                    )
                nc.vector.tensor_copy(
                    out=xT[:, :, nt * 128:(nt + 1) * 128], in_=ps[:]
                )

            def qk_unit(j, qk_i, cch):
                # one chunk-group: 128 channels x 512 tokens of q^T or k^T
                sl = slice(cch * 512, (cch + 1) * 512)
                ps = psA.tile(
                    [128, 512], F32, tag="s", name=f"qk{j}_{qk_i}_{cch}"
                )
                for kc in range(KC):
                    nc.tensor.matmul(
                        ps[:],
                        lhsT=wqk[:, kc, j, qk_i, :],
                        rhs=xT[:, kc, sl],
                        start=(kc == 0),
                        stop=(kc == KC - 1),
                    )
                nc.vector.tensor_copy(out=qkT[:, j, qk_i, sl], in_=ps[:])

            def v_unit(t, cch):
                # one chunk-group of V = X @ W_v (natural layout);
                # cch 0 covers heads 0..7, cch 1 heads 8..11
                sl = (slice(0, 512), slice(512, C))[cch]
                hs = (slice(0, 8), slice(8, NH))[cch]
                w = 512 if cch == 0 else C - 512
                ps = psA.tile([128, w], F32, tag="s", name=f"v{t}_{cch}")
                for kc in range(KC):
                    nc.tensor.matmul(
                        ps[:],
                        lhsT=xT[:, kc, t * 128:(t + 1) * 128],
                        rhs=wv[:, kc, sl],
                        start=(kc == 0),
                        stop=(kc == KC - 1),
                    )
                nc.vector.tensor_copy(
                    out=V[:, t, hs, 0:D],
                    in_=ps[:].rearrange("p (h d) -> p h d", d=D),
                )

            def qk_unit0(qk_i, q0):
                # pair-0 sub-chunk (256 cols): interleaves between transposes
                # so qk^T starts as soon as the first x tiles land
                ps = psA.tile(
                    [128, 256], F32, tag="s", name=f"qk0_{qk_i}_{q0}"
                )
                for kc in range(KC):
                    nc.tensor.matmul(
                        ps[:],
                        lhsT=wqk[:, kc, 0, qk_i, :],
                        rhs=xT[:, kc, q0:q0 + 256],
                        start=(kc == 0),
                        stop=(kc == KC - 1),
                    )
                nc.vector.tensor_copy(
                    out=qkT[:, 0, qk_i, q0:q0 + 256], in_=ps[:]
                )

            for nt in range(NT):
                transpose_tile(nt)
                if nt % 2 == 1:
                    q0 = (nt // 2) * 256
                    qk_unit0(0, q0)
                    qk_unit0(1, q0)
            for t in range(4):
                v_unit(t, 0)
                v_unit(t, 1)

            # Filler units, keyed by (pair, t) iteration of phase B; emitted
            # after that iteration's PVs.  Pair 0 hosts V(t4..7); pair j
            # hosts pair j+1's qk units.
            fill = {}

            def add_fill(j, t, fn):
                fill.setdefault((j, t), []).append(fn)

            for t in range(4, NT):
                it = 2 * (t - 4)
                add_fill(0, it, (lambda tt: lambda: v_unit(tt, 0))(t))
                add_fill(0, it + 1, (lambda tt: lambda: v_unit(tt, 1))(t))
            for j in range(1, NP):
                slots = ((0, 4), (0, 5), (0, 6), (0, 7)) if j == 1 else \
                    ((j - 1, 1), (j - 1, 3), (j - 1, 5), (j - 1, 7))
                for u, (jj, tt) in enumerate(slots):
                    qk_i, cch = divmod(u, 2)
                    add_fill(
                        jj, tt,
                        (lambda a, b, c: lambda: qk_unit(a, b, c))(j, qk_i, cch),
                    )

            # ---- Phase B: attention --------------------------------------
            OTs = {}

            def s_pair(j, t):
                # both heads' S tiles; with PAIR_S the two matmuls per chunk
                # sit on disjoint PE row-groups (K=64 at partitions 0/64) and
                # run concurrently
                S0 = psA.tile([128, N], F32, tag="s", name=f"s{2 * j}_{t}")
                S1 = psA.tile([128, N], F32, tag="s", name=f"s{2 * j + 1}_{t}")
                if PAIR_S:
                    order = [(cch, pb, S) for cch in range(2)
                             for pb, S in ((0, S0), (64, S1))]
                else:
                    order = [(cch, pb, S) for pb, S in ((0, S0), (64, S1))
                             for cch in range(2)]
                for cch, pb, S in order:
                    sl = slice(cch * 512, (cch + 1) * 512)
                    nc.tensor.matmul(
                        S[:, sl],
                        lhsT=qkT[pb:pb + 64, j, 1, t * 128:(t + 1) * 128],
                        rhs=qkT[pb:pb + 64, j, 0, sl],
                        start=True,
                        stop=True,
                    )
                pTs = []
                for hh, S in ((0, S0), (1, S1)):
                    pT = ptpool.tile(
                        [128, N], BF16, tag="pt", name=f"p{2 * j + hh}_{t}"
                    )
                    nc.scalar.activation(
                        out=pT[:], in_=S[:], func=Exp, scale=SCALE
                    )
                    pTs.append(pT)
                return pTs

            def s_single(h, t):
                j, hh = divmod(h, 2)
                pb = hh * 64
                S = psA.tile([128, N], F32, tag="s", name=f"s{h}_{t}")
                for cch in range(2):
                    sl = slice(cch * 512, (cch + 1) * 512)
                    nc.tensor.matmul(
                        S[:, sl],
                        lhsT=qkT[pb:pb + 64, j, 1, t * 128:(t + 1) * 128],
                        rhs=qkT[pb:pb + 64, j, 0, sl],
                        start=True,
                        stop=True,
                    )
                pT = ptpool.tile([128, N], BF16, tag="pt", name=f"p{h}_{t}")
                nc.scalar.activation(out=pT[:], in_=S[:], func=Exp, scale=SCALE)
                return pT

            def pv_matmul(h, t, pT):
                if t == 0:
                    OTs[h] = psO.tile([D + 1, N], F32, tag="ot", name=f"ot{h}")
                OT = OTs[h]
                for cch in range(2):
                    sl = slice(cch * 512, (cch + 1) * 512)
                    nc.tensor.matmul(
                        OT[:, sl],
                        lhsT=V[:, t, h, 0:D + 1],
                        rhs=pT[:, sl],
                        start=(t == 0),
                        stop=(t == NT - 1),
                    )

            def normalize(h, last=False):
                # copy O^T out fast (frees the PSUM slot), then recip the
                # denominator row (from a partition-0 tile — the custom-DVE
                # recip mis-executes on HW with a partition-base-64 input),
                # broadcast, multiply.
                j, hh = divmod(h, 2)
                pb = hh * 64
                OT = OTs.pop(h)
                if last:
                    # drain path: skip the ou stage, work straight from PSUM
                    # in column halves so proj closes unlock ASAP.  Den
                    # copies ride on ScalarE (idle here) so DVE's recip/mul
                    # chain isn't self-delayed.
                    den = bcpool.tile([1, N], F32, tag="den", name=f"d{h}")
                    bc = bcpool.tile([64, N], F32, tag="bc", name=f"b{h}")
                    for cch in range(2):
                        sl = slice(cch * 512, (cch + 1) * 512)
                        nc.scalar.copy(den[:, sl], OT[D:D + 1, sl])
                    for cch in range(2):
                        sl = slice(cch * 512, (cch + 1) * 512)
                        nc.vector.reciprocal_approx_fast(
                            out=bc[0:1, sl], in_=den[:, sl]
                        )
                        nc.gpsimd.partition_broadcast(
                            bc[:, sl], bc[0:1, sl], channels=64
                        )
                        nc.vector.tensor_mul(
                            out=ctxT[pb:pb + 64, j, sl],
                            in0=OT[0:D, sl],
                            in1=bc[:, sl],
                        )
                    return
                ou = oupool.tile([D + 1, N], F32, tag="ou", name=f"ou{h}")
                nc.vector.tensor_copy(out=ou[:], in_=OT[:])
                den = bcpool.tile([1, N], F32, tag="den", name=f"d{h}")
                nc.vector.tensor_copy(out=den[:], in_=ou[D:D + 1, :])
                bc = bcpool.tile([64, N], F32, tag="bc", name=f"b{h}")
                nc.vector.reciprocal_approx_fast(out=bc[0:1, :], in_=den[:])
                nc.gpsimd.partition_broadcast(bc[:], bc[0:1, :], channels=64)
                nc.vector.tensor_mul(
                    out=ctxT[pb:pb + 64, j, :], in0=ou[0:D, :], in1=bc[:]
                )

            # ---- Phase C: projection; first groups bridge the drain --------
            # group g = output tile nt=g//2, columns cch=g%2 (384 wide); one
            # [128, 2, 512] PSUM tile hosts two groups in bank-aligned halves.
            proj_ps = {}
            proj_ob = {}

            def proj_open(g, pool):
                nt, cch = divmod(g, 2)
                sl = slice(cch * 384, (cch + 1) * 384)
                if g % 2 == 0:
                    tag = "s" if pool is psA else "ot"
                    proj_ps[g // 2] = pool.tile(
                        [128, 2, 512], F32, tag=tag, name=f"pj{g // 2}"
                    )
                ps = proj_ps[g // 2][:, g % 2, 0:384]
                for kc in range(KC - 1):
                    nc.tensor.matmul(
                        ps,
                        lhsT=ctxT[:, kc, nt * 128:(nt + 1) * 128],
                        rhs=wp[:, kc, sl],
                        start=(kc == 0),
                        stop=False,
                    )

            def proj_close(g):
                nt, cch = divmod(g, 2)
                sl = slice(cch * 384, (cch + 1) * 384)
                ps = proj_ps[g // 2][:, g % 2, 0:384]
                nc.tensor.matmul(
                    ps,
                    lhsT=ctxT[:, KC - 1, nt * 128:(nt + 1) * 128],
                    rhs=wp[:, KC - 1, sl],
                    start=False,
                    stop=True,
                )
                # bias-add fused into the output cast on DVE (no bias matmul).
                # Even groups go via an ScalarE PSUM->SBUF copy first so the
                # PSUM slot recycles without waiting on the DVE queue.  Both
                # column halves land in one full-width ob tile so the y DMA
                # writes whole contiguous DRAM rows (1536B bursts, not 768B).
                if cch == 0:
                    proj_ob[nt] = outpool.tile(
                        [128, C], BF16, tag="ob", name=f"ob{nt}"
                    )
                ob = proj_ob[nt]
                if g % 2 == 0:
                    tmp = outpool.tile(
                        [128, 384], F32, tag="tmp", bufs=3, name=f"tm{g}"
                    )
                    nc.scalar.copy(tmp[:], ps)
                    src = tmp[:]
                else:
                    src = ps
                nc.vector.scalar_tensor_tensor(
                    out=ob[:, sl], in0=src, scalar=1.0, in1=bias_bc[:, sl],
                    op0=mybir.AluOpType.mult, op1=mybir.AluOpType.add,
                )
                if cch == 1:
                    eng = (nc.sync, nc.scalar, nc.gpsimd)[nt % 3]
                    eng.dma_start(
                        out=y[nt * 128:(nt + 1) * 128, :], in_=ob[:]
                    )

            # bridge: h11's exp + normalize chain runs under proj partials

            pending = deque()

            def pop_pv(last=False):
                h, t, pT = pending.popleft()
                pv_matmul(h, t, pT)
                if t == NT - 1:
                    normalize(h, last=last)

            for j in range(5):          # paired pairs 0..4, PV lag 3 iters
                for t in range(NT):
                    pT0, pT1 = s_pair(j, t)
                    pending.append((2 * j, t, pT0))
                    pending.append((2 * j + 1, t, pT1))
                    while len(pending) > 6:
                        pop_pv()
                    for fn in fill.get((j, t), ()):
                        fn()
            for h in (10, 11):          # pair 5 serial, PV lag 1
                for t in range(NT):
                    pT = s_single(h, t)
                    pending.append((h, t, pT))
                    while len(pending) > 1:
                        pop_pv()

            # psO slot 1 still holds OT(h11) until the normalize muls read
            # it, so the groups borrowing that slot (6,7) open only after
            # the first closes
            proj_open(0, psA)
            proj_open(1, psA)
            pop_pv(last=True)           # PV(h11, t7) + normalize(h11)
            proj_open(2, psO)
            proj_open(3, psO)
            proj_open(4, psA)
            proj_open(5, psA)
            for g in range(4):
                proj_close(g)
            proj_open(6, psO)
            proj_open(7, psO)
            for g in range(4, 8):
                proj_close(g)
            for g in range(8, 16):
                proj_open(g, psA if (g // 2) % 2 == 0 else psO)
                proj_close(g)

    nc.finalize()
    return nc


def _get_nc():
    if "nc" not in _CACHE:
        _CACHE["nc"] = _build_nc()
    return _CACHE["nc"]


def _make_in_maps(x, w_qkv, w_proj, b_proj):
    B = x.shape[0]
    xb = np.ascontiguousarray(x.reshape(B, N, C).astype(np.float32))
    w_qkv = np.ascontiguousarray(w_qkv.astype(np.float32))
    w_proj = np.ascontiguousarray(w_proj.astype(np.float32))
    bp = np.ascontiguousarray(b_proj.reshape(1, C).astype(np.float32))
    return [
        {"x": xb[b], "w_qkv": w_qkv, "w_proj": w_proj, "b_proj": bp}
        for b in range(B)
    ]


def _run(in_maps, **kwargs):
    from concourse.bass_utils import run_bass_kernel_spmd

    nc = _get_nc()
    return run_bass_kernel_spmd(
        nc, in_maps, core_ids=list(range(NCORES)), **kwargs
    )


def kernel(x, w_qkv, w_proj, b_proj):
    B, H, W, _ = x.shape
    res = _run(_make_in_maps(x, w_qkv, w_proj, b_proj))
    out = np.stack(
        [np.asarray(res.results[b]["y"], dtype=np.float32) for b in range(B)]
    )
    return out.reshape(B, H, W, C)


# revision 33
# speedup vs baseline: 1.2434x; 1.0005x over previous
"""Trainium2 Bass kernel for a 12-head self-attention block.

Reference computation (per batch b of 8):
    qkv = x @ w_qkv                      # (1024, 2304)
    q, k, v per head (12 heads, d=64)
    attn = softmax(q k^T / sqrt(64))
    ctx  = attn @ v                      # (1024, 768)
    y    = ctx @ w_proj + b_proj

Sharding: data parallel over the batch dim — batch b runs on core b.
Each core gets the full weights and its own x slice; no collectives.

Performance structure (v3): the PE clock is HAM-gated — any idle gap
drops it from 2.4 GHz to 1.2 GHz for ~30 us, so the PE instruction
stream is kept gapless:
  - S matmuls have K=64; a head pair's k^T/q^T live on disjoint
    partition halves, so the pair's two S matmuls issue back-to-back
    with row tile_positions (0,*)/(64,*) and run CONCURRENTLY on
    disjoint PE row-groups (~2x S throughput).  Pairs 0-4 run paired;
    pair 5 runs its heads serially so the final drain is single-head.
  - PV matmuls lag their S by three iterations; qk^T / V chunk-groups
    interleave as filler so the PE never waits on ScalarE's exp (the
    only exp engine, ~2.2us per paired iteration).
  - The projection's per-tile accumulation groups open early (bias +
    kc0..4) to bridge the last head's normalize-chain drain; kc=5
    closes once the last ctx tile lands.  Proj tiles alternate between
    the two PSUM pools so four groups are always in flight.
  - Softmax denominators ride as a ones-column in V (free: PV streams
    cost N columns regardless of M=65).
  - y is written to DRAM as bf16 (halves writeback; host casts back).
"""

import numpy as np

N = 1024          # tokens per batch (32*32)
C = 768           # model dim
NH = 12           # heads
D = 64            # head dim
NT = N // 128     # 8 token tiles
KC = C // 128     # 6 contraction tiles
NP = NH // 2      # 6 head pairs
SCALE = D ** -0.5
NCORES = 8
PAIR_S = False    # paired issue thrashes the PE weight buffer (+100ns/MM)

_CACHE = {}


def _build_nc():
    import concourse.bass as bass
    import concourse.tile as tile
    from concourse import bacc, mybir
    from concourse.masks import make_identity
    from collections import deque

    F32 = mybir.dt.float32
    BF16 = mybir.dt.bfloat16
    Exp = mybir.ActivationFunctionType.Exp

    nc = bacc.Bacc(None, target_bir_lowering=False)
    x = nc.declare_dram_parameter("x", [N, C], F32, isOutput=False)
    wqkv = nc.declare_dram_parameter("w_qkv", [C, 3 * C], F32, isOutput=False)
    wproj = nc.declare_dram_parameter("w_proj", [C, C], F32, isOutput=False)
    bproj = nc.declare_dram_parameter("b_proj", [1, C], F32, isOutput=False)
    y = nc.declare_dram_parameter("y", [N, C], BF16, isOutput=True)

    with tile.TileContext(nc) as tc:
        from contextlib import ExitStack

        with ExitStack() as ctx:
            persist = ctx.enter_context(tc.tile_pool(name="persist", bufs=1))
            xT = persist.tile([128, KC, N], BF16)           # X^T (c, n)
            wqk = persist.tile([128, KC, NP, 2, 128], BF16)  # W_q|W_k per pair
            wv = persist.tile([128, KC, C], BF16)
            V = persist.tile([128, NT, NH, D + 2], BF16)    # v + ones col
            wp = persist.tile([128, KC, C], BF16)
            ctxT = persist.tile([128, KC, N], BF16)         # normalized ctx^T
            qkT = persist.tile([128, NP, 2, N], BF16)       # all pairs q^T/k^T
            ident = persist.tile([128, 128], BF16)
            ones_f32 = persist.tile([128, 128], F32)
            bias_sb = persist.tile([1, C], F32)
            bias_bc = persist.tile([128, C], F32)   # bias broadcast to 128 rows

            make_identity(nc, ident)
            nc.vector.memset(ones_f32[:], 1.0)
            for _t in range(NT):
                # ones written in pairs (4-byte chunks): lone 2-byte strided
                # writes are not safe on the compute engines
                nc.any.tensor_copy(
                    out=V[:, _t, :, D:D + 2],
                    in_=ones_f32[:, 0:2 * NH].rearrange(
                        "p (h two) -> p h two", two=2
                    ),
                )

            # ---- input DMAs ------------------------------------------------
            # x rows on the two HWDGE queues; weights on SWDGE (casts f32 ->
            # bf16 in flight).  SWDGE order = need order: pair-0 qk weights,
            # then wv (first PV needs V(t0)), remaining pairs, wproj.
            # wqkv viewed as [p, kc, {q,k,v}, pair, 128]
            wqkv_v = wqkv.rearrange(
                "(kc p) (three pair c) -> p kc three pair c",
                p=128, three=3, c=128,
            )
            for qk_i in range(2):
                nc.gpsimd.dma_start(
                    out=wqk[:, :, 0, qk_i], in_=wqkv_v[:, :, qk_i, 0]
                )

            # x rows on the two HWDGE queues (SWDGE is busy with weights and
            # would delay the first tiles)
            xpool = ctx.enter_context(tc.tile_pool(name="xload", bufs=8))
            xin = []
            for nt in range(NT):
                xt_in = xpool.tile([128, C], F32, tag="x")
                r = slice(nt * 128, (nt + 1) * 128)
                nc.sync.dma_start(out=xt_in[:, 0:384], in_=x[r, 0:384])
                nc.scalar.dma_start(out=xt_in[:, 384:C], in_=x[r, 384:C])
                xin.append(xt_in)
            for kc in range(KC):
                nc.gpsimd.dma_start(
                    out=wv[:, kc, :],
                    in_=wqkv[kc * 128:(kc + 1) * 128, 2 * C:3 * C],
                )
            for j in range(1, NP):
                for qk_i in range(2):
                    nc.gpsimd.dma_start(
                        out=wqk[:, :, j, qk_i], in_=wqkv_v[:, :, qk_i, j]
                    )
            for kc in range(KC):
                nc.gpsimd.dma_start(
                    out=wp[:, kc, :],
                    in_=wproj[kc * 128:(kc + 1) * 128, :],
                )
            nc.gpsimd.dma_start(out=bias_sb[:], in_=bproj[:])
            nc.gpsimd.partition_broadcast(bias_bc[:], bias_sb[:], channels=128)

            # ---- PSUM pools (8 banks total, both pools 2 x 4KB slots) ------
            psA = ctx.enter_context(
                tc.tile_pool(name="psA", bufs=2, space="PSUM")
            )
            psO = ctx.enter_context(
                tc.tile_pool(name="psO", bufs=2, space="PSUM")
            )
            ptpool = ctx.enter_context(tc.tile_pool(name="pt", bufs=8))
            oupool = ctx.enter_context(tc.tile_pool(name="ou", bufs=2))
            bcpool = ctx.enter_context(tc.tile_pool(name="bc", bufs=2))
            outpool = ctx.enter_context(tc.tile_pool(name="out", bufs=6))

            # ---- Phase A: X^T transposes + pair-0 qk^T + V(t0..t3) ---------
            # x is cast f32->bf16 on DVE first so the PE transposes run at
            # 1 cycle/row instead of f32's 2
            xbpool = ctx.enter_context(tc.tile_pool(name="xb", bufs=3))

            def transpose_tile(nt):
                # cast per x-half so the first transposes start as soon as
                # the sync-queue half lands
                xb = xbpool.tile([128, C], BF16, tag="xb", name=f"xb{nt}")
                nc.vector.tensor_copy(out=xb[:, 0:384], in_=xin[nt][:, 0:384])
                nc.vector.tensor_copy(out=xb[:, 384:C], in_=xin[nt][:, 384:C])
                ps = psA.tile([128, KC, 128], BF16, tag="s", name=f"tp{nt}")
                for kc in range(KC):
                    nc.tensor.transpose(
                        ps[:, kc, :],
                        xb[:, kc * 128:(kc + 1) * 128],
                        ident[:],
                    )
                nc.vector.tensor_copy(
                    out=xT[:, :, nt * 128:(nt + 1) * 128], in_=ps[:]
                )

            def qk_unit(j, qk_i, cch):
                # one chunk-group: 128 channels x 512 tokens of q^T or k^T
                sl = slice(cch * 512, (cch + 1) * 512)
                ps = psA.tile(
                    [128, 512], F32, tag="s", name=f"qk{j}_{qk_i}_{cch}"
                )
                for kc in range(KC):
                    nc.tensor.matmul(
                        ps[:],
                        lhsT=wqk[:, kc, j, qk_i, :],
                        rhs=xT[:, kc, sl],
                        start=(kc == 0),
                        stop=(kc == KC - 1),
                    )
                nc.vector.tensor_copy(out=qkT[:, j, qk_i, sl], in_=ps[:])

            def v_unit(t, cch):
                # one chunk-group of V = X @ W_v (natural layout);
                # cch 0 covers heads 0..7, cch 1 heads 8..11
                sl = (slice(0, 512), slice(512, C))[cch]
                hs = (slice(0, 8), slice(8, NH))[cch]
                w = 512 if cch == 0 else C - 512
                ps = psA.tile([128, w], F32, tag="s", name=f"v{t}_{cch}")
                for kc in range(KC):
                    nc.tensor.matmul(
                        ps[:],
                        lhsT=xT[:, kc, t * 128:(t + 1) * 128],
                        rhs=wv[:, kc, sl],
                        start=(kc == 0),
                        stop=(kc == KC - 1),
                    )
                nc.vector.tensor_copy(
                    out=V[:, t, hs, 0:D],
                    in_=ps[:].rearrange("p (h d) -> p h d", d=D),
                )

            def qk_unit0(qk_i, q0):
                # pair-0 sub-chunk (256 cols): interleaves between transposes
                # so qk^T starts as soon as the first x tiles land
                ps = psA.tile(
                    [128, 256], F32, tag="s", name=f"qk0_{qk_i}_{q0}"
                )
                for kc in range(KC):
                    nc.tensor.matmul(
                        ps[:],
                        lhsT=wqk[:, kc, 0, qk_i, :],
                        rhs=xT[:, kc, q0:q0 + 256],
                        start=(kc == 0),
                        stop=(kc == KC - 1),
                    )
                nc.vector.tensor_copy(
                    out=qkT[:, 0, qk_i, q0:q0 + 256], in_=ps[:]
                )

            for nt in range(NT):
                transpose_tile(nt)
                if nt % 2 == 1:
                    q0 = (nt // 2) * 256
                    qk_unit0(0, q0)
                    qk_unit0(1, q0)
            for t in range(4):
                v_unit(t, 0)
                v_unit(t, 1)

            # Filler units, keyed by (pair, t) iteration of phase B; emitted
            # after that iteration's PVs.  Pair 0 hosts V(t4..7); pair j
            # hosts pair j+1's qk units.
            fill = {}

            def add_fill(j, t, fn):
                fill.setdefault((j, t), []).append(fn)

            for t in range(4, NT):
                it = 2 * (t - 4)
                add_fill(0, it, (lambda tt: lambda: v_unit(tt, 0))(t))
                add_fill(0, it + 1, (lambda tt: lambda: v_unit(tt, 1))(t))
            for j in range(1, NP):
                slots = ((0, 4), (0, 5), (0, 6), (0, 7)) if j == 1 else \
                    ((j - 1, 1), (j - 1, 3), (j - 1, 5), (j - 1, 7))
                for u, (jj, tt) in enumerate(slots):
                    qk_i, cch = divmod(u, 2)
                    add_fill(
                        jj, tt,
                        (lambda a, b, c: lambda: qk_unit(a, b, c))(j, qk_i, cch),
                    )

            # ---- Phase B: attention --------------------------------------
            OTs = {}

            def s_pair(j, t):
                # both heads' S tiles; with PAIR_S the two matmuls per chunk
                # sit on disjoint PE row-groups (K=64 at partitions 0/64) and
                # run concurrently
                S0 = psA.tile([128, N], F32, tag="s", name=f"s{2 * j}_{t}")
                S1 = psA.tile([128, N], F32, tag="s", name=f"s{2 * j + 1}_{t}")
                if PAIR_S:
                    order = [(cch, pb, S) for cch in range(2)
                             for pb, S in ((0, S0), (64, S1))]
                else:
                    order = [(cch, pb, S) for pb, S in ((0, S0), (64, S1))
                             for cch in range(2)]
                for cch, pb, S in order:
                    sl = slice(cch * 512, (cch + 1) * 512)
                    nc.tensor.matmul(
                        S[:, sl],
                        lhsT=qkT[pb:pb + 64, j, 1, t * 128:(t + 1) * 128],
                        rhs=qkT[pb:pb + 64, j, 0, sl],
                        start=True,
                        stop=True,
                    )
                pTs = []
                for hh, S in ((0, S0), (1, S1)):
                    pT = ptpool.tile(
                        [128, N], BF16, tag="pt", name=f"p{2 * j + hh}_{t}"
                    )
                    nc.scalar.activation(
                        out=pT[:], in_=S[:], func=Exp, scale=SCALE
                    )
                    pTs.append(pT)
                return pTs

            def s_single(h, t):
                j, hh = divmod(h, 2)
                pb = hh * 64
                S = psA.tile([128, N], F32, tag="s", name=f"s{h}_{t}")
                for cch in range(2):
                    sl = slice(cch * 512, (cch + 1) * 512)
                    nc.tensor.matmul(
                        S[:, sl],
                        lhsT=qkT[pb:pb + 64, j, 1, t * 128:(t + 1) * 128],
                        rhs=qkT[pb:pb + 64, j, 0, sl],
                        start=True,
                        stop=True,
                    )
                pT = ptpool.tile([128, N], BF16, tag="pt", name=f"p{h}_{t}")
                nc.scalar.activation(out=pT[:], in_=S[:], func=Exp, scale=SCALE)
                return pT

            def pv_matmul(h, t, pT):
                if t == 0:
                    OTs[h] = psO.tile([D + 1, N], F32, tag="ot", name=f"ot{h}")
                OT = OTs[h]
                for cch in range(2):
                    sl = slice(cch * 512, (cch + 1) * 512)
                    nc.tensor.matmul(
                        OT[:, sl],
                        lhsT=V[:, t, h, 0:D + 1],
                        rhs=pT[:, sl],
                        start=(t == 0),
                        stop=(t == NT - 1),
                    )

            def normalize(h, last=False):
                # copy O^T out fast (frees the PSUM slot), then recip the
                # denominator row (from a partition-0 tile — the custom-DVE
                # recip mis-executes on HW with a partition-base-64 input),
                # broadcast, multiply.
                j, hh = divmod(h, 2)
                pb = hh * 64
                OT = OTs.pop(h)
                if last:
                    # drain path: skip the ou stage, work straight from PSUM
                    # in column halves so proj closes unlock ASAP.  Den
                    # copies ride on ScalarE (idle here) so DVE's recip/mul
                    # chain isn't self-delayed.
                    den = bcpool.tile([1, N], F32, tag="den", name=f"d{h}")
                    bc = bcpool.tile([64, N], F32, tag="bc", name=f"b{h}")
                    for cch in range(2):
                        sl = slice(cch * 512, (cch + 1) * 512)
                        nc.scalar.copy(den[:, sl], OT[D:D + 1, sl])
                    for cch in range(2):
                        sl = slice(cch * 512, (cch + 1) * 512)
                        nc.vector.reciprocal_approx_fast(
                            out=bc[0:1, sl], in_=den[:, sl]
                        )
                        nc.gpsimd.partition_broadcast(
                            bc[:, sl], bc[0:1, sl], channels=64
                        )
                        nc.vector.tensor_mul(
                            out=ctxT[pb:pb + 64, j, sl],
                            in0=OT[0:D, sl],
                            in1=bc[:, sl],
                        )
                    return
                ou = oupool.tile([D + 1, N], F32, tag="ou", name=f"ou{h}")
                nc.vector.tensor_copy(out=ou[:], in_=OT[:])
                den = bcpool.tile([1, N], F32, tag="den", name=f"d{h}")
                nc.vector.tensor_copy(out=den[:], in_=ou[D:D + 1, :])
                bc = bcpool.tile([64, N], F32, tag="bc", name=f"b{h}")
                nc.vector.reciprocal_approx_fast(out=bc[0:1, :], in_=den[:])
                nc.gpsimd.partition_broadcast(bc[:], bc[0:1, :], channels=64)
                nc.vector.tensor_mul(
                    out=ctxT[pb:pb + 64, j, :], in0=ou[0:D, :], in1=bc[:]
                )

            # ---- Phase C: projection; first groups bridge the drain --------
            # group g = output tile nt=g//2, columns cch=g%2 (384 wide); one
            # [128, 2, 512] PSUM tile hosts two groups in bank-aligned halves.
            proj_ps = {}
            proj_ob = {}

            def proj_open(g, pool):
                nt, cch = divmod(g, 2)
                sl = slice(cch * 384, (cch + 1) * 384)
                if g % 2 == 0:
                    tag = "s" if pool is psA else "ot"
                    proj_ps[g // 2] = pool.tile(
                        [128, 2, 512], F32, tag=tag, name=f"pj{g // 2}"
                    )
                ps = proj_ps[g // 2][:, g % 2, 0:384]
                for kc in range(KC - 1):
                    nc.tensor.matmul(
                        ps,
                        lhsT=ctxT[:, kc, nt * 128:(nt + 1) * 128],
                        rhs=wp[:, kc, sl],
                        start=(kc == 0),
                        stop=False,
                    )

            def proj_close(g):
                nt, cch = divmod(g, 2)
                sl = slice(cch * 384, (cch + 1) * 384)
                ps = proj_ps[g // 2][:, g % 2, 0:384]
                nc.tensor.matmul(
                    ps,
                    lhsT=ctxT[:, KC - 1, nt * 128:(nt + 1) * 128],
                    rhs=wp[:, KC - 1, sl],
                    start=False,
                    stop=True,
                )
                # bias-add fused into the output cast on DVE (no bias matmul).
                # Even groups go via an ScalarE PSUM->SBUF copy first so the
                # PSUM slot recycles without waiting on the DVE queue.  Both
                # column halves land in one full-width ob tile so the y DMA
                # writes whole contiguous DRAM rows (1536B bursts, not 768B).
                if cch == 0:
                    proj_ob[nt] = outpool.tile(
                        [128, C], BF16, tag="ob", name=f"ob{nt}"
                    )
                ob = proj_ob[nt]
                if g % 2 == 0:
                    tmp = outpool.tile(
                        [128, 384], F32, tag="tmp", bufs=3, name=f"tm{g}"
                    )
                    nc.scalar.copy(tmp[:], ps)
                    src = tmp[:]
                else:
                    src = ps
                nc.vector.scalar_tensor_tensor(
                    out=ob[:, sl], in0=src, scalar=1.0, in1=bias_bc[:, sl],
                    op0=mybir.AluOpType.mult, op1=mybir.AluOpType.add,
                )
                if cch == 1:
                    # two pieces on two queues: one dma_start's descriptors
                    # ride ONE DMA channel (~30GB/s), so a single full-row
                    # transfer strands the last tile ~6.5us past PE-end
                    r = slice(nt * 128, (nt + 1) * 128)
                    e0, e1 = ((nc.sync, nc.scalar), (nc.scalar, nc.gpsimd),
                              (nc.gpsimd, nc.sync))[nt % 3]
                    e0.dma_start(out=y[r, 0:384], in_=ob[:, 0:384])
                    e1.dma_start(out=y[r, 384:C], in_=ob[:, 384:C])

            # bridge: h11's exp + normalize chain runs under proj partials

            pending = deque()

            def pop_pv(last=False):
                h, t, pT = pending.popleft()
                pv_matmul(h, t, pT)
                if t == NT - 1:
                    normalize(h, last=last)

            for j in range(5):          # paired pairs 0..4, PV lag 3 iters
                for t in range(NT):
                    pT0, pT1 = s_pair(j, t)
                    pending.append((2 * j, t, pT0))
                    pending.append((2 * j + 1, t, pT1))
                    while len(pending) > 6:
                        pop_pv()
                    for fn in fill.get((j, t), ()):
                        fn()
            for h in (10, 11):          # pair 5 serial, PV lag 1
                for t in range(NT):
                    pT = s_single(h, t)
                    pending.append((h, t, pT))
                    while len(pending) > 1:
                        pop_pv()

            # psO slot 1 still holds OT(h11) until the normalize muls read
            # it, so the groups borrowing that slot (6,7) open only after
            # the first closes
            proj_open(0, psA)
            proj_open(1, psA)
            pop_pv(last=True)           # PV(h11, t7) + normalize(h11)
            proj_open(2, psO)
            proj_open(3, psO)
            proj_open(4, psA)
            proj_open(5, psA)
            for g in range(4):
                proj_close(g)
            proj_open(6, psO)
            proj_open(7, psO)
            for g in range(4, 8):
                proj_close(g)
            for g in range(8, 16):
                proj_open(g, psA if (g // 2) % 2 == 0 else psO)
                proj_close(g)

    nc.finalize()
    return nc


def _get_nc():
    if "nc" not in _CACHE:
        _CACHE["nc"] = _build_nc()
    return _CACHE["nc"]


def _make_in_maps(x, w_qkv, w_proj, b_proj):
    B = x.shape[0]
    xb = np.ascontiguousarray(x.reshape(B, N, C).astype(np.float32))
    w_qkv = np.ascontiguousarray(w_qkv.astype(np.float32))
    w_proj = np.ascontiguousarray(w_proj.astype(np.float32))
    bp = np.ascontiguousarray(b_proj.reshape(1, C).astype(np.float32))
    return [
        {"x": xb[b], "w_qkv": w_qkv, "w_proj": w_proj, "b_proj": bp}
        for b in range(B)
    ]


def _run(in_maps, **kwargs):
    from concourse.bass_utils import run_bass_kernel_spmd

    nc = _get_nc()
    return run_bass_kernel_spmd(
        nc, in_maps, core_ids=list(range(NCORES)), **kwargs
    )


def kernel(x, w_qkv, w_proj, b_proj):
    B, H, W, _ = x.shape
    res = _run(_make_in_maps(x, w_qkv, w_proj, b_proj))
    out = np.stack(
        [np.asarray(res.results[b]["y"], dtype=np.float32) for b in range(B)]
    )
    return out.reshape(B, H, W, C)


# revision 34
# speedup vs baseline: 1.2435x; 1.0000x over previous
"""Trainium2 Bass kernel for a 12-head self-attention block.

Reference computation (per batch b of 8):
    qkv = x @ w_qkv                      # (1024, 2304)
    q, k, v per head (12 heads, d=64)
    attn = softmax(q k^T / sqrt(64))
    ctx  = attn @ v                      # (1024, 768)
    y    = ctx @ w_proj + b_proj

Sharding: data parallel over the batch dim — batch b runs on core b.
Each core gets the full weights and its own x slice; no collectives.

Performance structure (v3): the PE clock is HAM-gated — any idle gap
drops it from 2.4 GHz to 1.2 GHz for ~30 us, so the PE instruction
stream is kept gapless:
  - S matmuls have K=64; a head pair's k^T/q^T live on disjoint
    partition halves, so the pair's two S matmuls issue back-to-back
    with row tile_positions (0,*)/(64,*) and run CONCURRENTLY on
    disjoint PE row-groups (~2x S throughput).  Pairs 0-4 run paired;
    pair 5 runs its heads serially so the final drain is single-head.
  - PV matmuls lag their S by three iterations; qk^T / V chunk-groups
    interleave as filler so the PE never waits on ScalarE's exp (the
    only exp engine, ~2.2us per paired iteration).
  - The projection's per-tile accumulation groups open early (bias +
    kc0..4) to bridge the last head's normalize-chain drain; kc=5
    closes once the last ctx tile lands.  Proj tiles alternate between
    the two PSUM pools so four groups are always in flight.
  - Softmax denominators ride as a ones-column in V (free: PV streams
    cost N columns regardless of M=65).
  - y is written to DRAM as bf16 (halves writeback; host casts back).
"""

import numpy as np

N = 1024          # tokens per batch (32*32)
C = 768           # model dim
NH = 12           # heads
D = 64            # head dim
NT = N // 128     # 8 token tiles
KC = C // 128     # 6 contraction tiles
NP = NH // 2      # 6 head pairs
SCALE = D ** -0.5
NCORES = 8
PAIR_S = False    # paired issue thrashes the PE weight buffer (+100ns/MM)

_CACHE = {}


def _build_nc():
    import concourse.bass as bass
    import concourse.tile as tile
    from concourse import bacc, mybir
    from concourse.masks import make_identity
    from collections import deque

    F32 = mybir.dt.float32
    BF16 = mybir.dt.bfloat16
    Exp = mybir.ActivationFunctionType.Exp

    nc = bacc.Bacc(None, target_bir_lowering=False)
    x = nc.declare_dram_parameter("x", [N, C], F32, isOutput=False)
    wqkv = nc.declare_dram_parameter("w_qkv", [C, 3 * C], F32, isOutput=False)
    wproj = nc.declare_dram_parameter("w_proj", [C, C], F32, isOutput=False)
    bproj = nc.declare_dram_parameter("b_proj", [1, C], F32, isOutput=False)
    y = nc.declare_dram_parameter("y", [N, C], BF16, isOutput=True)

    with tile.TileContext(nc) as tc:
        from contextlib import ExitStack

        with ExitStack() as ctx:
            persist = ctx.enter_context(tc.tile_pool(name="persist", bufs=1))
            xT = persist.tile([128, KC, N], BF16)           # X^T (c, n)
            wqk = persist.tile([128, KC, NP, 2, 128], BF16)  # W_q|W_k per pair
            wv = persist.tile([128, KC, C], BF16)
            V = persist.tile([128, NT, NH, D + 2], BF16)    # v + ones col
            wp = persist.tile([128, KC, C], BF16)
            ctxT = persist.tile([128, KC, N], BF16)         # normalized ctx^T
            qkT = persist.tile([128, NP, 2, N], BF16)       # all pairs q^T/k^T
            ident = persist.tile([128, 128], BF16)
            ones_f32 = persist.tile([128, 128], F32)
            bias_sb = persist.tile([1, C], F32)
            bias_bc = persist.tile([128, C], F32)   # bias broadcast to 128 rows

            make_identity(nc, ident)
            nc.vector.memset(ones_f32[:], 1.0)
            for _t in range(NT):
                # ones written in pairs (4-byte chunks): lone 2-byte strided
                # writes are not safe on the compute engines
                nc.any.tensor_copy(
                    out=V[:, _t, :, D:D + 2],
                    in_=ones_f32[:, 0:2 * NH].rearrange(
                        "p (h two) -> p h two", two=2
                    ),
                )

            # ---- input DMAs ------------------------------------------------
            # x rows on the two HWDGE queues; weights on SWDGE (casts f32 ->
            # bf16 in flight).  SWDGE order = need order: pair-0 qk weights,
            # then wv (first PV needs V(t0)), remaining pairs, wproj.
            # wqkv viewed as [p, kc, {q,k,v}, pair, 128]
            wqkv_v = wqkv.rearrange(
                "(kc p) (three pair c) -> p kc three pair c",
                p=128, three=3, c=128,
            )
            for qk_i in range(2):
                nc.gpsimd.dma_start(
                    out=wqk[:, :, 0, qk_i], in_=wqkv_v[:, :, qk_i, 0]
                )

            # x rows on the two HWDGE queues (SWDGE is busy with weights and
            # would delay the first tiles)
            xpool = ctx.enter_context(tc.tile_pool(name="xload", bufs=8))
            xin = []
            for nt in range(NT):
                xt_in = xpool.tile([128, C], F32, tag="x")
                r = slice(nt * 128, (nt + 1) * 128)
                nc.sync.dma_start(out=xt_in[:, 0:384], in_=x[r, 0:384])
                nc.scalar.dma_start(out=xt_in[:, 384:C], in_=x[r, 384:C])
                xin.append(xt_in)
            for kc in range(KC):
                nc.gpsimd.dma_start(
                    out=wv[:, kc, :],
                    in_=wqkv[kc * 128:(kc + 1) * 128, 2 * C:3 * C],
                )
            for j in range(1, NP):
                for qk_i in range(2):
                    nc.gpsimd.dma_start(
                        out=wqk[:, :, j, qk_i], in_=wqkv_v[:, :, qk_i, j]
                    )
            for kc in range(KC):
                nc.gpsimd.dma_start(
                    out=wp[:, kc, :],
                    in_=wproj[kc * 128:(kc + 1) * 128, :],
                )
            nc.gpsimd.dma_start(out=bias_sb[:], in_=bproj[:])
            nc.gpsimd.partition_broadcast(bias_bc[:], bias_sb[:], channels=128)

            # ---- PSUM pools (8 banks total, both pools 2 x 4KB slots) ------
            psA = ctx.enter_context(
                tc.tile_pool(name="psA", bufs=2, space="PSUM")
            )
            psO = ctx.enter_context(
                tc.tile_pool(name="psO", bufs=2, space="PSUM")
            )
            ptpool = ctx.enter_context(tc.tile_pool(name="pt", bufs=8))
            oupool = ctx.enter_context(tc.tile_pool(name="ou", bufs=2))
            bcpool = ctx.enter_context(tc.tile_pool(name="bc", bufs=2))
            outpool = ctx.enter_context(tc.tile_pool(name="out", bufs=6))

            # ---- Phase A: X^T transposes + pair-0 qk^T + V(t0..t3) ---------
            # x is cast f32->bf16 on DVE first so the PE transposes run at
            # 1 cycle/row instead of f32's 2
            xbpool = ctx.enter_context(tc.tile_pool(name="xb", bufs=3))

            def transpose_tile(nt):
                # cast per x-half so the first transposes start as soon as
                # the sync-queue half lands
                xb = xbpool.tile([128, C], BF16, tag="xb", name=f"xb{nt}")
                nc.vector.tensor_copy(out=xb[:, 0:384], in_=xin[nt][:, 0:384])
                nc.vector.tensor_copy(out=xb[:, 384:C], in_=xin[nt][:, 384:C])
                ps = psA.tile([128, KC, 128], BF16, tag="s", name=f"tp{nt}")
                for kc in range(KC):
                    nc.tensor.transpose(
                        ps[:, kc, :],
                        xb[:, kc * 128:(kc + 1) * 128],
                        ident[:],
                    )
                nc.vector.tensor_copy(
                    out=xT[:, :, nt * 128:(nt + 1) * 128], in_=ps[:]
                )

            def qk_unit(j, qk_i, cch):
                # one chunk-group: 128 channels x 512 tokens of q^T or k^T
                sl = slice(cch * 512, (cch + 1) * 512)
                ps = psA.tile(
                    [128, 512], F32, tag="s", name=f"qk{j}_{qk_i}_{cch}"
                )
                for kc in range(KC):
                    nc.tensor.matmul(
                        ps[:],
                        lhsT=wqk[:, kc, j, qk_i, :],
                        rhs=xT[:, kc, sl],
                        start=(kc == 0),
                        stop=(kc == KC - 1),
                    )
                nc.vector.tensor_copy(out=qkT[:, j, qk_i, sl], in_=ps[:])

            def v_unit(t, cch):
                # one chunk-group of V = X @ W_v (natural layout);
                # cch 0 covers heads 0..7, cch 1 heads 8..11
                sl = (slice(0, 512), slice(512, C))[cch]
                hs = (slice(0, 8), slice(8, NH))[cch]
                w = 512 if cch == 0 else C - 512
                ps = psA.tile([128, w], F32, tag="s", name=f"v{t}_{cch}")
                for kc in range(KC):
                    nc.tensor.matmul(
                        ps[:],
                        lhsT=xT[:, kc, t * 128:(t + 1) * 128],
                        rhs=wv[:, kc, sl],
                        start=(kc == 0),
                        stop=(kc == KC - 1),
                    )
                nc.vector.tensor_copy(
                    out=V[:, t, hs, 0:D],
                    in_=ps[:].rearrange("p (h d) -> p h d", d=D),
                )

            def qk_unit0(qk_i, q0):
                # pair-0 sub-chunk (256 cols): interleaves between transposes
                # so qk^T starts as soon as the first x tiles land
                ps = psA.tile(
                    [128, 256], F32, tag="s", name=f"qk0_{qk_i}_{q0}"
                )
                for kc in range(KC):
                    nc.tensor.matmul(
                        ps[:],
                        lhsT=wqk[:, kc, 0, qk_i, :],
                        rhs=xT[:, kc, q0:q0 + 256],
                        start=(kc == 0),
                        stop=(kc == KC - 1),
                    )
                nc.vector.tensor_copy(
                    out=qkT[:, 0, qk_i, q0:q0 + 256], in_=ps[:]
                )

            for nt in range(NT):
                transpose_tile(nt)
                if nt % 2 == 1:
                    q0 = (nt // 2) * 256
                    qk_unit0(0, q0)
                    qk_unit0(1, q0)
            for t in range(4):
                v_unit(t, 0)
                v_unit(t, 1)

            # Filler units, keyed by (pair, t) iteration of phase B; emitted
            # after that iteration's PVs.  Pair 0 hosts V(t4..7); pair j
            # hosts pair j+1's qk units.
            fill = {}

            def add_fill(j, t, fn):
                fill.setdefault((j, t), []).append(fn)

            for t in range(4, NT):
                it = 2 * (t - 4)
                add_fill(0, it, (lambda tt: lambda: v_unit(tt, 0))(t))
                add_fill(0, it + 1, (lambda tt: lambda: v_unit(tt, 1))(t))
            for j in range(1, NP):
                slots = ((0, 4), (0, 5), (0, 6), (0, 7)) if j == 1 else \
                    ((j - 1, 1), (j - 1, 3), (j - 1, 5), (j - 1, 7))
                for u, (jj, tt) in enumerate(slots):
                    qk_i, cch = divmod(u, 2)
                    add_fill(
                        jj, tt,
                        (lambda a, b, c: lambda: qk_unit(a, b, c))(j, qk_i, cch),
                    )

            # ---- Phase B: attention --------------------------------------
            OTs = {}

            def s_pair(j, t):
                # both heads' S tiles; with PAIR_S the two matmuls per chunk
                # sit on disjoint PE row-groups (K=64 at partitions 0/64) and
                # run concurrently
                S0 = psA.tile([128, N], F32, tag="s", name=f"s{2 * j}_{t}")
                S1 = psA.tile([128, N], F32, tag="s", name=f"s{2 * j + 1}_{t}")
                if PAIR_S:
                    order = [(cch, pb, S) for cch in range(2)
                             for pb, S in ((0, S0), (64, S1))]
                else:
                    order = [(cch, pb, S) for pb, S in ((0, S0), (64, S1))
                             for cch in range(2)]
                for cch, pb, S in order:
                    sl = slice(cch * 512, (cch + 1) * 512)
                    nc.tensor.matmul(
                        S[:, sl],
                        lhsT=qkT[pb:pb + 64, j, 1, t * 128:(t + 1) * 128],
                        rhs=qkT[pb:pb + 64, j, 0, sl],
                        start=True,
                        stop=True,
                    )
                pTs = []
                for hh, S in ((0, S0), (1, S1)):
                    pT = ptpool.tile(
                        [128, N], BF16, tag="pt", name=f"p{2 * j + hh}_{t}"
                    )
                    nc.scalar.activation(
                        out=pT[:], in_=S[:], func=Exp, scale=SCALE
                    )
                    pTs.append(pT)
                return pTs

            def s_single(h, t):
                j, hh = divmod(h, 2)
                pb = hh * 64
                S = psA.tile([128, N], F32, tag="s", name=f"s{h}_{t}")
                for cch in range(2):
                    sl = slice(cch * 512, (cch + 1) * 512)
                    nc.tensor.matmul(
                        S[:, sl],
                        lhsT=qkT[pb:pb + 64, j, 1, t * 128:(t + 1) * 128],
                        rhs=qkT[pb:pb + 64, j, 0, sl],
                        start=True,
                        stop=True,
                    )
                pT = ptpool.tile([128, N], BF16, tag="pt", name=f"p{h}_{t}")
                nc.scalar.activation(out=pT[:], in_=S[:], func=Exp, scale=SCALE)
                return pT

            def pv_matmul(h, t, pT):
                if t == 0:
                    OTs[h] = psO.tile([D + 1, N], F32, tag="ot", name=f"ot{h}")
                OT = OTs[h]
                for cch in range(2):
                    sl = slice(cch * 512, (cch + 1) * 512)
                    nc.tensor.matmul(
                        OT[:, sl],
                        lhsT=V[:, t, h, 0:D + 1],
                        rhs=pT[:, sl],
                        start=(t == 0),
                        stop=(t == NT - 1),
                    )

            def normalize(h, last=False):
                # copy O^T out fast (frees the PSUM slot), then recip the
                # denominator row (from a partition-0 tile — the custom-DVE
                # recip mis-executes on HW with a partition-base-64 input),
                # broadcast, multiply.
                j, hh = divmod(h, 2)
                pb = hh * 64
                OT = OTs.pop(h)
                if last:
                    # drain path: skip the ou stage, work straight from PSUM
                    # in column halves so proj closes unlock ASAP.  Den
                    # copies ride on ScalarE (idle here) so DVE's recip/mul
                    # chain isn't self-delayed.
                    den = bcpool.tile([1, N], F32, tag="den", name=f"d{h}")
                    bc = bcpool.tile([64, N], F32, tag="bc", name=f"b{h}")
                    for cch in range(2):
                        sl = slice(cch * 512, (cch + 1) * 512)
                        nc.scalar.copy(den[:, sl], OT[D:D + 1, sl])
                    for cch in range(2):
                        sl = slice(cch * 512, (cch + 1) * 512)
                        nc.vector.reciprocal_approx_fast(
                            out=bc[0:1, sl], in_=den[:, sl]
                        )
                        nc.gpsimd.partition_broadcast(
                            bc[:, sl], bc[0:1, sl], channels=64
                        )
                        nc.vector.tensor_mul(
                            out=ctxT[pb:pb + 64, j, sl],
                            in0=OT[0:D, sl],
                            in1=bc[:, sl],
                        )
                    return
                ou = oupool.tile([D + 1, N], F32, tag="ou", name=f"ou{h}")
                nc.vector.tensor_copy(out=ou[:], in_=OT[:])
                den = bcpool.tile([1, N], F32, tag="den", name=f"d{h}")
                nc.vector.tensor_copy(out=den[:], in_=ou[D:D + 1, :])
                bc = bcpool.tile([64, N], F32, tag="bc", name=f"b{h}")
                nc.vector.reciprocal_approx_fast(out=bc[0:1, :], in_=den[:])
                nc.gpsimd.partition_broadcast(bc[:], bc[0:1, :], channels=64)
                nc.vector.tensor_mul(
                    out=ctxT[pb:pb + 64, j, :], in0=ou[0:D, :], in1=bc[:]
                )

            # ---- Phase C: projection; first groups bridge the drain --------
            # group g = output tile nt=g//2, columns cch=g%2 (384 wide); one
            # [128, 2, 512] PSUM tile hosts two groups in bank-aligned halves.
            proj_ps = {}
            proj_ob = {}

            def proj_open(g, pool):
                nt, cch = divmod(g, 2)
                sl = slice(cch * 384, (cch + 1) * 384)
                if g % 2 == 0:
                    tag = "s" if pool is psA else "ot"
                    proj_ps[g // 2] = pool.tile(
                        [128, 2, 512], F32, tag=tag, name=f"pj{g // 2}"
                    )
                ps = proj_ps[g // 2][:, g % 2, 0:384]
                for kc in range(KC - 1):
                    nc.tensor.matmul(
                        ps,
                        lhsT=ctxT[:, kc, nt * 128:(nt + 1) * 128],
                        rhs=wp[:, kc, sl],
                        start=(kc == 0),
                        stop=False,
                    )

            def proj_close(g):
                nt, cch = divmod(g, 2)
                sl = slice(cch * 384, (cch + 1) * 384)
                ps = proj_ps[g // 2][:, g % 2, 0:384]
                nc.tensor.matmul(
                    ps,
                    lhsT=ctxT[:, KC - 1, nt * 128:(nt + 1) * 128],
                    rhs=wp[:, KC - 1, sl],
                    start=False,
                    stop=True,
                )
                # bias-add fused into the output cast on DVE (no bias matmul).
                # Even groups go via an ScalarE PSUM->SBUF copy first so the
                # PSUM slot recycles without waiting on the DVE queue.  Both
                # column halves land in one full-width ob tile so the y DMA
                # writes whole contiguous DRAM rows (1536B bursts, not 768B).
                if cch == 0:
                    proj_ob[nt] = outpool.tile(
                        [128, C], BF16, tag="ob", name=f"ob{nt}"
                    )
                ob = proj_ob[nt]
                if g % 2 == 0:
                    tmp = outpool.tile(
                        [128, 384], F32, tag="tmp", bufs=3, name=f"tm{g}"
                    )
                    nc.scalar.copy(tmp[:], ps)
                    src = tmp[:]
                else:
                    src = ps
                nc.vector.scalar_tensor_tensor(
                    out=ob[:, sl], in0=src, scalar=1.0, in1=bias_bc[:, sl],
                    op0=mybir.AluOpType.mult, op1=mybir.AluOpType.add,
                )
                if cch == 1:
                    # two pieces on the two HWDGE queues: one dma_start's
                    # descriptors ride ONE DMA channel, and SWDGE (gpsimd)
                    # generates descriptors at ~50ns/row (~6.5us per tile),
                    # so y stays off the SWDGE ring entirely
                    r = slice(nt * 128, (nt + 1) * 128)
                    nc.sync.dma_start(out=y[r, 0:384], in_=ob[:, 0:384])
                    nc.scalar.dma_start(out=y[r, 384:C], in_=ob[:, 384:C])

            # bridge: h11's exp + normalize chain runs under proj partials

            pending = deque()

            def pop_pv(last=False):
                h, t, pT = pending.popleft()
                pv_matmul(h, t, pT)
                if t == NT - 1:
                    normalize(h, last=last)

            for j in range(5):          # paired pairs 0..4, PV lag 3 iters
                for t in range(NT):
                    pT0, pT1 = s_pair(j, t)
                    pending.append((2 * j, t, pT0))
                    pending.append((2 * j + 1, t, pT1))
                    while len(pending) > 6:
                        pop_pv()
                    for fn in fill.get((j, t), ()):
                        fn()
            for h in (10, 11):          # pair 5 serial, PV lag 1
                for t in range(NT):
                    pT = s_single(h, t)
                    pending.append((h, t, pT))
                    while len(pending) > 1:
                        pop_pv()

            # psO slot 1 still holds OT(h11) until the normalize muls read
            # it, so the groups borrowing that slot (6,7) open only after
            # the first closes
            proj_open(0, psA)
            proj_open(1, psA)
            pop_pv(last=True)           # PV(h11, t7) + normalize(h11)
            proj_open(2, psO)
            proj_open(3, psO)
            proj_open(4, psA)
            proj_open(5, psA)
            for g in range(4):
                proj_close(g)
            proj_open(6, psO)
            proj_open(7, psO)
            for g in range(4, 8):
                proj_close(g)
            for g in range(8, 16):
                proj_open(g, psA if (g // 2) % 2 == 0 else psO)
                proj_close(g)

    nc.finalize()
    return nc


def _get_nc():
    if "nc" not in _CACHE:
        _CACHE["nc"] = _build_nc()
    return _CACHE["nc"]


def _make_in_maps(x, w_qkv, w_proj, b_proj):
    B = x.shape[0]
    xb = np.ascontiguousarray(x.reshape(B, N, C).astype(np.float32))
    w_qkv = np.ascontiguousarray(w_qkv.astype(np.float32))
    w_proj = np.ascontiguousarray(w_proj.astype(np.float32))
    bp = np.ascontiguousarray(b_proj.reshape(1, C).astype(np.float32))
    return [
        {"x": xb[b], "w_qkv": w_qkv, "w_proj": w_proj, "b_proj": bp}
        for b in range(B)
    ]


def _run(in_maps, **kwargs):
    from concourse.bass_utils import run_bass_kernel_spmd

    nc = _get_nc()
    return run_bass_kernel_spmd(
        nc, in_maps, core_ids=list(range(NCORES)), **kwargs
    )


def kernel(x, w_qkv, w_proj, b_proj):
    B, H, W, _ = x.shape
    res = _run(_make_in_maps(x, w_qkv, w_proj, b_proj))
    out = np.stack(
        [np.asarray(res.results[b]["y"], dtype=np.float32) for b in range(B)]
    )
    return out.reshape(B, H, W, C)
